# revision 9
# baseline (speedup 1.0000x reference)
"""Trainium2 Bass kernel for non-uniform 3D Catmull-Rom spline interpolation.

Problem: x [131072, 3] query points, knots [3, 48], grid [48,48,48,16]
-> out [131072, 16].

Strategy (data-parallel over the batch across 8 cores):
  Host: pad the grid to [50,50,50,16]; build a replicated gather table
    gtab[(p0, q0, r0), 1040] whose 4160B rows hold the full 4x4x4x16
    neighborhood gp[p0:p0+4, q0:q0+4, r0:r0+4, :] (1024 f32) plus the 12
    knot-window values [tm1,t0,t1,t2] per dim (keyed by p0/q0/r0), padded
    to 1040 f32.  A query's entire working set is then ONE contiguous 4KB+
    row -> one [128,1]-index indirect DMA per 128-query tile (the only
    reliably-ordered indirect-DMA shape on TRN2 SWDGE; >=512B descriptors
    run at full DMA rate).
  Device (per core, 16384 queries = 128 tiles of 128 queries on partitions):
    - searchsorted per dim via 47 fused (x >= knot_j) + acc ops (batched
      over all queries)
    - row index = idx0*47*47 + idx1*47 + idx2 -> gidx [128,128] int32
    - per supertile of G tiles: G gathers, Catmull-Rom weights from the
      embedded knot windows (batched [128,G] DVE ops), then per-tile
      contraction k -> j -> i with tensor_scalar / scalar_tensor_tensor
      fused multiply-adds using per-partition scalars (per-query weights)
"""

import numpy as np

import concourse.bass as bass
import concourse.bacc as bacc
import concourse.tile as tile
from concourse import mybir
from concourse.bass_utils import run_bass_kernel_spmd

# ---- problem constants (hardcoded per harness contract) ----
B, D, N, V = 131072, 3, 48, 16
NCORES = 8
BS = B // NCORES  # 16384 queries per core
P = 128           # partitions
T = BS // P       # 128 tiles of 128 queries
NP = N + 2        # 50 (padded grid extent)
NI = N - 1        # 47 interval starts (idx in [0, 46])
NB = 4 * 4 * 4 * V       # 1024 f32 neighborhood per query
ROW = NB + 16            # + [3 dims x 4 knot vals] + 4 pad = 1040 f32 (4160B)
NROWS = NI * NI * NI     # 103823 gather-table rows
G = 16            # tiles per supertile (weight-batching granularity)
NST = T // G      # supertiles

F32 = mybir.dt.float32
I32 = mybir.dt.int32
OP = mybir.AluOpType

DEBUG = False
REPS = 1  # phase-B repetitions (differential HW timing; harness uses 1)


def _emit_kernel(nc: bass.Bass):
    xkn = nc.dram_tensor("xkn", [P, D * T + D * N], F32, kind="ExternalInput")
    gtab = nc.dram_tensor("gtab", [NROWS, ROW], F32, kind="ExternalInput")
    outb = nc.dram_tensor("outb", [P, T * V], F32, kind="ExternalOutput")
    if DEBUG:
        dbg_gidx = nc.dram_tensor("dbg_gidx", [P, T], I32, kind="ExternalOutput")
        dbg_nb = nc.dram_tensor("dbg_nb", [P, G * ROW], F32, kind="ExternalOutput")
        dbg_w = nc.dram_tensor("dbg_w", [P, D * 4 * G], F32, kind="ExternalOutput")

    with tile.TileContext(nc) as tc:
        from contextlib import ExitStack

        with ExitStack() as ctx:
            singles = ctx.enter_context(tc.tile_pool(name="singles", bufs=1))
            nbpool = ctx.enter_context(tc.tile_pool(name="nb", bufs=2 * G))
            wpool = ctx.enter_context(tc.tile_pool(name="wp", bufs=2))
            accpool = ctx.enter_context(tc.tile_pool(name="acc", bufs=4))

            # ---------------- load inputs (one DMA) ----------------
            sb_xk = singles.tile([P, D * T + D * N], F32, tag="sb_xk", name="sb_xk")
            nc.sync.dma_start(out=sb_xk[:], in_=xkn[:])
            sb_x = sb_xk[:, : D * T].rearrange("p (d t) -> p d t", d=D)
            sb_kn = sb_xk[:, D * T :].rearrange("p (d n) -> p d n", d=D)

            # ---------------- phase A: searchsorted + row index ----------------
            idx_f = [
                singles.tile([P, T], F32, tag=f"idx{d}", name=f"idx{d}")
                for d in range(D)
            ]
            for d in range(D):
                xd = sb_x[:, d, :]
                acc = idx_f[d]
                nc.vector.tensor_scalar(
                    out=acc[:], in0=xd, scalar1=sb_kn[:, d, 1:2], scalar2=None,
                    op0=OP.is_ge,
                )
                for j in range(2, N):
                    nc.vector.scalar_tensor_tensor(
                        out=acc[:], in0=xd, scalar=sb_kn[:, d, j : j + 1],
                        in1=acc[:], op0=OP.is_ge, op1=OP.add,
                    )
                nc.vector.tensor_scalar(
                    out=acc[:], in0=acc[:], scalar1=float(NI - 1), scalar2=None,
                    op0=OP.min,
                )

            # base row = idx0*NI*NI + idx1*NI + idx2
            basef = singles.tile([P, T], F32, tag="basef", name="basef")
            nc.vector.scalar_tensor_tensor(
                out=basef[:], in0=idx_f[1][:], scalar=float(NI), in1=idx_f[2][:],
                op0=OP.mult, op1=OP.add,
            )
            nc.vector.scalar_tensor_tensor(
                out=basef[:], in0=idx_f[0][:], scalar=float(NI * NI), in1=basef[:],
                op0=OP.mult, op1=OP.add,
            )
            gidx = singles.tile([P, T], I32, tag="gidx", name="gidx")
            nc.vector.tensor_copy(out=gidx[:], in_=basef[:])
            if DEBUG:
                nc.sync.dma_start(out=dbg_gidx[:], in_=gidx[:])

            # ---------------- phase B: gather + weights + contract ----------
            outbuf = singles.tile([P, T * V], F32, tag="outbuf", name="outbuf")
            for s in range(NST * REPS):
                s = s % NST
                nbs = [
                    nbpool.tile([P, ROW], F32, tag="nb", name=f"nb_{s}_{tl}")
                    for tl in range(G)
                ]
                for tl in range(G):
                    t = s * G + tl
                    nc.gpsimd.indirect_dma_start(
                        out=nbs[tl][:],
                        out_offset=None,
                        in_=gtab[:],
                        in_offset=bass.IndirectOffsetOnAxis(
                            ap=gidx[:, t : t + 1], axis=0
                        ),
                    )
                if DEBUG and s == 0:
                    for tl in range(G):
                        nc.sync.dma_start(
                            out=dbg_nb[:, tl * ROW : (tl + 1) * ROW], in_=nbs[tl][:]
                        )

                # collect the embedded knot windows: kvst[p, tl*16 + (d*4+c)]
                kvst = wpool.tile([P, G * 16], F32, tag="kvst", name="kvst")
                for tl in range(G):
                    nc.vector.tensor_copy(
                        out=kvst[:, tl * 16 : (tl + 1) * 16], in_=nbs[tl][:, NB:]
                    )

                # weights per dim, batched over the supertile: [P, G] ops
                w = [
                    [
                        wpool.tile([P, G], F32, tag=f"w{d}{c}", name=f"w{d}{c}")
                        for c in range(4)
                    ]
                    for d in range(D)
                ]
                scr = [
                    wpool.tile([P, G], F32, tag=f"scr{i}", name=f"scr{i}")
                    for i in range(6)
                ]
                u_t, u2_t, u3_t, s0, s1, s2 = scr
                for d in range(D):
                    xd = sb_x[:, d, s * G : (s + 1) * G]
                    kbase = kvst[:]

                    def kvv(c, _kb=kbase, _d=d):
                        return bass.AP(
                            kvst.tensor,
                            _kb.offset + _d * 4 + c,
                            [_kb.ap[0], [16, G]],
                        )

                    tm1, t0, t1, t2 = kvv(0), kvv(1), kvv(2), kvv(3)
                    dt = s0
                    nc.vector.tensor_sub(out=dt[:], in0=t1, in1=t0)
                    xm = s1
                    nc.vector.tensor_sub(out=xm[:], in0=xd, in1=t0)  # x - t0
                    rcp = s2
                    nc.vector.reciprocal(out=rcp[:], in_=dt[:])
                    nc.vector.tensor_mul(out=u_t[:], in0=xm[:], in1=rcp[:])  # u
                    nc.vector.tensor_mul(out=u2_t[:], in0=u_t[:], in1=u_t[:])
                    nc.vector.tensor_mul(out=u3_t[:], in0=u2_t[:], in1=u_t[:])
                    a_t = s1
                    nc.vector.tensor_sub(out=a_t[:], in0=t1, in1=tm1)
                    nc.vector.reciprocal(out=a_t[:], in_=a_t[:])
                    nc.vector.tensor_mul(out=a_t[:], in0=a_t[:], in1=dt[:])
                    b_t = s2
                    nc.vector.tensor_sub(out=b_t[:], in0=t2, in1=t0)
                    nc.vector.reciprocal(out=b_t[:], in_=b_t[:])
                    nc.vector.tensor_mul(out=b_t[:], in0=b_t[:], in1=dt[:])
                    # h00 = 2u3 - 3u2 + 1 ; h10 = u3 - 2u2 + u ; h11 = u3 - u2
                    h00 = w[d][1]
                    nc.vector.tensor_scalar(
                        out=h00[:], in0=u2_t[:], scalar1=-3.0, scalar2=1.0,
                        op0=OP.mult, op1=OP.add,
                    )
                    nc.vector.scalar_tensor_tensor(
                        out=h00[:], in0=u3_t[:], scalar=2.0, in1=h00[:],
                        op0=OP.mult, op1=OP.add,
                    )
                    h10 = w[d][0]
                    nc.vector.tensor_scalar(
                        out=h10[:], in0=u2_t[:], scalar1=-2.0, scalar2=None,
                        op0=OP.mult,
                    )
                    nc.vector.tensor_add(out=h10[:], in0=h10[:], in1=u_t[:])
                    nc.vector.tensor_add(out=h10[:], in0=h10[:], in1=u3_t[:])
                    h11 = w[d][3]
                    nc.vector.tensor_sub(out=h11[:], in0=u3_t[:], in1=u2_t[:])
                    p1 = w[d][0]
                    nc.vector.tensor_mul(out=p1[:], in0=h10[:], in1=a_t[:])
                    p2 = w[d][3]
                    nc.vector.tensor_mul(out=p2[:], in0=h11[:], in1=b_t[:])
                    # w2 = (p1 - h00) + 1 ; w1 = h00 - p2 ; w0 = -p1 ; w3 = p2
                    nc.vector.tensor_sub(out=w[d][2][:], in0=p1[:], in1=h00[:])
                    nc.vector.tensor_scalar(
                        out=w[d][2][:], in0=w[d][2][:], scalar1=1.0, scalar2=None,
                        op0=OP.add,
                    )
                    nc.vector.tensor_sub(out=w[d][1][:], in0=h00[:], in1=p2[:])
                    nc.vector.tensor_scalar(
                        out=w[d][0][:], in0=p1[:], scalar1=-1.0, scalar2=None,
                        op0=OP.mult,
                    )
                if DEBUG and s == 0:
                    for d in range(D):
                        for c in range(4):
                            nc.sync.dma_start(
                                out=dbg_w[
                                    :, (d * 4 + c) * G : (d * 4 + c + 1) * G
                                ],
                                in_=w[d][c][:],
                            )

                # contraction per tile: layout [i(4)x256, j(4)x64, k(4)x16, v(16)]
                for tl in range(G):
                    t = s * G + tl
                    tview = nbs[tl][:]
                    acc1 = accpool.tile([P, 256], F32, tag="acc1", name="acc1")
                    for k in range(4):
                        kslice = bass.AP(
                            nbs[tl].tensor,
                            tview.offset + k * V,
                            [tview.ap[0], [64, 16], [1, V]],
                        )
                        if k == 0:
                            nc.vector.tensor_scalar(
                                out=acc1[:], in0=kslice,
                                scalar1=w[2][k][:, tl : tl + 1],
                                scalar2=None, op0=OP.mult,
                            )
                        else:
                            nc.vector.scalar_tensor_tensor(
                                out=acc1[:], in0=kslice,
                                scalar=w[2][k][:, tl : tl + 1],
                                in1=acc1[:], op0=OP.mult, op1=OP.add,
                            )
                    acc2 = accpool.tile([P, 64], F32, tag="acc2", name="acc2")
                    for j in range(4):
                        jslice = bass.AP(
                            acc1.tensor,
                            acc1[:].offset + j * V,
                            [acc1[:].ap[0], [64, 4], [1, V]],
                        )
                        if j == 0:
                            nc.vector.tensor_scalar(
                                out=acc2[:], in0=jslice,
                                scalar1=w[1][j][:, tl : tl + 1],
                                scalar2=None, op0=OP.mult,
                            )
                        else:
                            nc.vector.scalar_tensor_tensor(
                                out=acc2[:], in0=jslice,
                                scalar=w[1][j][:, tl : tl + 1],
                                in1=acc2[:], op0=OP.mult, op1=OP.add,
                            )
                    oslice = outbuf[:, t * V : (t + 1) * V]
                    for i in range(4):
                        islice = acc2[:, i * V : (i + 1) * V]
                        if i == 0:
                            nc.vector.tensor_scalar(
                                out=oslice, in0=islice,
                                scalar1=w[0][i][:, tl : tl + 1],
                                scalar2=None, op0=OP.mult,
                            )
                        else:
                            nc.vector.scalar_tensor_tensor(
                                out=oslice, in0=islice,
                                scalar=w[0][i][:, tl : tl + 1],
                                in1=oslice, op0=OP.mult, op1=OP.add,
                            )
            nc.sync.dma_start(out=outb[:], in_=outbuf[:])
    return nc


_NC_CACHE = None


def _get_nc():
    global _NC_CACHE
    if _NC_CACHE is None:
        nc = bacc.Bacc(None, target_bir_lowering=False)
        _emit_kernel(nc)
        nc.compile()
        _NC_CACHE = nc
    return _NC_CACHE


_PREP_CACHE = {}


def _host_prep(knots, grid):
    key = (id(knots), id(grid))
    if key in _PREP_CACHE:
        return _PREP_CACHE[key]
    from numpy.lib.stride_tricks import sliding_window_view

    gp = np.asarray(grid, dtype=np.float32)
    for ax in range(3):
        lo = 2.0 * np.take(gp, 0, axis=ax) - np.take(gp, 1, axis=ax)
        hi = 2.0 * np.take(gp, -1, axis=ax) - np.take(gp, -2, axis=ax)
        gp = np.concatenate(
            [np.expand_dims(lo, ax), gp, np.expand_dims(hi, ax)], axis=ax
        )
    # neighborhood rows [47,47,47, 4i,4j,4k,16v]
    sw = sliding_window_view(gp, (4, 4, 4), axis=(0, 1, 2))
    nbr = sw.transpose(0, 1, 2, 4, 5, 6, 3).reshape(NROWS, NB)
    # knot windows per dim: ktw[d][i] = padded_knots[d][i:i+4]
    kn = np.asarray(knots, dtype=np.float32)
    tp = np.concatenate(
        [2.0 * kn[:, :1] - kn[:, 1:2], kn, 2.0 * kn[:, -1:] - kn[:, -2:-1]], axis=1
    )
    ktw = sliding_window_view(tp, 4, axis=1)  # [3, 47, 4]
    gtab = np.empty((NROWS, ROW), dtype=np.float32)
    gtab[:, :NB] = nbr
    kv = gtab[:, NB : NB + 12].reshape(NI, NI, NI, 3, 4)
    kv[:, :, :, 0, :] = ktw[0][:, None, None, :]
    kv[:, :, :, 1, :] = ktw[1][None, :, None, :]
    kv[:, :, :, 2, :] = ktw[2][None, None, :, :]
    gtab[:, NB + 12 :] = 0.0
    knr = np.ascontiguousarray(
        np.broadcast_to(kn.reshape(1, D * N), (P, D * N))
    ).astype(np.float32)
    _PREP_CACHE[key] = (gtab, knr)
    return gtab, knr


def kernel(x, knots, grid):
    x = np.asarray(x, dtype=np.float32)
    gtab, knr = _host_prep(knots, grid)
    nc = _get_nc()
    in_maps = []
    for c in range(NCORES):
        xs = x[c * BS : (c + 1) * BS]  # [BS, 3]
        xTc = np.ascontiguousarray(
            xs.reshape(T, P, D).transpose(1, 2, 0).reshape(P, D * T)
        )  # [p, d*T + t] = x[t*P + p, d]
        xknc = np.concatenate([xTc, knr], axis=1)
        in_maps.append({"xkn": xknc, "gtab": gtab})
    res = run_bass_kernel_spmd(nc, in_maps, core_ids=list(range(NCORES)))
    outs = []
    for c in range(NCORES):
        ob = res.results[c]["outb"].reshape(P, T, V)  # [p, t, v]
        outs.append(np.ascontiguousarray(ob.transpose(1, 0, 2)).reshape(BS, V))
    return np.concatenate(outs, axis=0)


# revision 14
# speedup vs baseline: 1.2455x; 1.2455x over previous
"""Trainium2 Bass kernel for non-uniform 3D Catmull-Rom spline interpolation.

Problem: x [131072, 3] query points, knots [3, 48], grid [48,48,48,16]
-> out [131072, 16].

Strategy (data-parallel over the batch across 8 cores):
  Host: pad the grid to [50,50,50,16]; build a replicated gather table
    gtab[(p0, q0, r0), 1040] whose 4160B rows hold the full 4x4x4x16
    neighborhood gp[p0:p0+4, q0:q0+4, r0:r0+4, :] (1024 f32) plus the 12
    knot-window values [tm1,t0,t1,t2] per dim (keyed by p0/q0/r0), padded
    to 1040 f32.  A query's entire working set is then ONE contiguous 4KB+
    row -> one [128,1]-index indirect DMA per 128-query tile (the only
    reliably-ordered indirect-DMA shape on TRN2 SWDGE; >=512B descriptors
    run at full DMA rate).
  Device (per core, 16384 queries = 128 tiles of 128 queries on partitions):
    - searchsorted per dim via 47 fused (x >= knot_j) + acc ops (batched
      over all queries)
    - row index = idx0*47*47 + idx1*47 + idx2 -> gidx [128,128] int32
    - per supertile of G tiles: G gathers, Catmull-Rom weights from the
      embedded knot windows (batched [128,G] DVE ops), then per-tile
      contraction k -> j -> i with tensor_scalar / scalar_tensor_tensor
      fused multiply-adds using per-partition scalars (per-query weights)
"""

import numpy as np

import concourse.bass as bass
import concourse.bacc as bacc
import concourse.tile as tile
from concourse import mybir
from concourse.bass_utils import run_bass_kernel_spmd

# ---- problem constants (hardcoded per harness contract) ----
B, D, N, V = 131072, 3, 48, 16
NCORES = 8
BS = B // NCORES  # 16384 queries per core
P = 128           # partitions
T = BS // P       # 128 tiles of 128 queries
NP = N + 2        # 50 (padded grid extent)
NI = N - 1        # 47 interval starts (idx in [0, 46])
NB = 4 * 4 * 4 * V       # 1024 f32 neighborhood per query
ROW = NB + 16            # + [3 dims x 4 knot vals] + 4 pad = 1040 f32 (4160B)
NROWS = NI * NI * NI     # 103823 gather-table rows
G = 16            # tiles per supertile (weight-batching granularity)
NST = T // G      # supertiles

F32 = mybir.dt.float32
I32 = mybir.dt.int32
OP = mybir.AluOpType

DEBUG = False
REPS = 1  # phase-B repetitions (differential HW timing; harness uses 1)


def _emit_kernel_general(nc: bass.Bass):
    xkn = nc.dram_tensor("xkn", [P, D * T + D * N], F32, kind="ExternalInput")
    gtab = nc.dram_tensor("gtab", [NROWS, ROW], F32, kind="ExternalInput")
    outb = nc.dram_tensor("outb", [P, T * V], F32, kind="ExternalOutput")
    if DEBUG:
        dbg_gidx = nc.dram_tensor("dbg_gidx", [P, T], I32, kind="ExternalOutput")
        dbg_nb = nc.dram_tensor("dbg_nb", [P, G * ROW], F32, kind="ExternalOutput")
        dbg_w = nc.dram_tensor("dbg_w", [P, D * 4 * G], F32, kind="ExternalOutput")

    with tile.TileContext(nc) as tc:
        from contextlib import ExitStack

        with ExitStack() as ctx:
            singles = ctx.enter_context(tc.tile_pool(name="singles", bufs=1))
            nbpool = ctx.enter_context(tc.tile_pool(name="nb", bufs=2 * G))
            wpool = ctx.enter_context(tc.tile_pool(name="wp", bufs=2))
            accpool = ctx.enter_context(tc.tile_pool(name="acc", bufs=4))

            # ---------------- load inputs (one DMA) ----------------
            sb_xk = singles.tile([P, D * T + D * N], F32, tag="sb_xk", name="sb_xk")
            nc.sync.dma_start(out=sb_xk[:], in_=xkn[:])
            sb_x = sb_xk[:, : D * T].rearrange("p (d t) -> p d t", d=D)
            sb_kn = sb_xk[:, D * T :].rearrange("p (d n) -> p d n", d=D)

            # ---------------- phase A: searchsorted + row index ----------------
            idx_f = [
                singles.tile([P, T], F32, tag=f"idx{d}", name=f"idx{d}")
                for d in range(D)
            ]
            for d in range(D):
                xd = sb_x[:, d, :]
                acc = idx_f[d]
                nc.vector.tensor_scalar(
                    out=acc[:], in0=xd, scalar1=sb_kn[:, d, 1:2], scalar2=None,
                    op0=OP.is_ge,
                )
                for j in range(2, N):
                    nc.vector.scalar_tensor_tensor(
                        out=acc[:], in0=xd, scalar=sb_kn[:, d, j : j + 1],
                        in1=acc[:], op0=OP.is_ge, op1=OP.add,
                    )
                nc.vector.tensor_scalar(
                    out=acc[:], in0=acc[:], scalar1=float(NI - 1), scalar2=None,
                    op0=OP.min,
                )

            # base row = idx0*NI*NI + idx1*NI + idx2
            basef = singles.tile([P, T], F32, tag="basef", name="basef")
            nc.vector.scalar_tensor_tensor(
                out=basef[:], in0=idx_f[1][:], scalar=float(NI), in1=idx_f[2][:],
                op0=OP.mult, op1=OP.add,
            )
            nc.vector.scalar_tensor_tensor(
                out=basef[:], in0=idx_f[0][:], scalar=float(NI * NI), in1=basef[:],
                op0=OP.mult, op1=OP.add,
            )
            gidx = singles.tile([P, T], I32, tag="gidx", name="gidx")
            nc.vector.tensor_copy(out=gidx[:], in_=basef[:])
            if DEBUG:
                nc.sync.dma_start(out=dbg_gidx[:], in_=gidx[:])

            # ---------------- phase B: gather + weights + contract ----------
            outbuf = singles.tile([P, T * V], F32, tag="outbuf", name="outbuf")
            for s in range(NST * REPS):
                s = s % NST
                nbs = [
                    nbpool.tile([P, ROW], F32, tag="nb", name=f"nb_{s}_{tl}")
                    for tl in range(G)
                ]
                for tl in range(G):
                    t = s * G + tl
                    nc.gpsimd.indirect_dma_start(
                        out=nbs[tl][:],
                        out_offset=None,
                        in_=gtab[:],
                        in_offset=bass.IndirectOffsetOnAxis(
                            ap=gidx[:, t : t + 1], axis=0
                        ),
                    )
                if DEBUG and s == 0:
                    for tl in range(G):
                        nc.sync.dma_start(
                            out=dbg_nb[:, tl * ROW : (tl + 1) * ROW], in_=nbs[tl][:]
                        )

                # collect the embedded knot windows: kvst[p, tl*16 + (d*4+c)]
                kvst = wpool.tile([P, G * 16], F32, tag="kvst", name="kvst")
                for tl in range(G):
                    nc.vector.tensor_copy(
                        out=kvst[:, tl * 16 : (tl + 1) * 16], in_=nbs[tl][:, NB:]
                    )

                # weights per dim, batched over the supertile: [P, G] ops
                w = [
                    [
                        wpool.tile([P, G], F32, tag=f"w{d}{c}", name=f"w{d}{c}")
                        for c in range(4)
                    ]
                    for d in range(D)
                ]
                scr = [
                    wpool.tile([P, G], F32, tag=f"scr{i}", name=f"scr{i}")
                    for i in range(6)
                ]
                u_t, u2_t, u3_t, s0, s1, s2 = scr
                for d in range(D):
                    xd = sb_x[:, d, s * G : (s + 1) * G]
                    kbase = kvst[:]

                    def kvv(c, _kb=kbase, _d=d):
                        return bass.AP(
                            kvst.tensor,
                            _kb.offset + _d * 4 + c,
                            [_kb.ap[0], [16, G]],
                        )

                    tm1, t0, t1, t2 = kvv(0), kvv(1), kvv(2), kvv(3)
                    dt = s0
                    nc.vector.tensor_sub(out=dt[:], in0=t1, in1=t0)
                    xm = s1
                    nc.vector.tensor_sub(out=xm[:], in0=xd, in1=t0)  # x - t0
                    rcp = s2
                    nc.vector.reciprocal(out=rcp[:], in_=dt[:])
                    nc.vector.tensor_mul(out=u_t[:], in0=xm[:], in1=rcp[:])  # u
                    nc.vector.tensor_mul(out=u2_t[:], in0=u_t[:], in1=u_t[:])
                    nc.vector.tensor_mul(out=u3_t[:], in0=u2_t[:], in1=u_t[:])
                    a_t = s1
                    nc.vector.tensor_sub(out=a_t[:], in0=t1, in1=tm1)
                    nc.vector.reciprocal(out=a_t[:], in_=a_t[:])
                    nc.vector.tensor_mul(out=a_t[:], in0=a_t[:], in1=dt[:])
                    b_t = s2
                    nc.vector.tensor_sub(out=b_t[:], in0=t2, in1=t0)
                    nc.vector.reciprocal(out=b_t[:], in_=b_t[:])
                    nc.vector.tensor_mul(out=b_t[:], in0=b_t[:], in1=dt[:])
                    # h00 = 2u3 - 3u2 + 1 ; h10 = u3 - 2u2 + u ; h11 = u3 - u2
                    h00 = w[d][1]
                    nc.vector.tensor_scalar(
                        out=h00[:], in0=u2_t[:], scalar1=-3.0, scalar2=1.0,
                        op0=OP.mult, op1=OP.add,
                    )
                    nc.vector.scalar_tensor_tensor(
                        out=h00[:], in0=u3_t[:], scalar=2.0, in1=h00[:],
                        op0=OP.mult, op1=OP.add,
                    )
                    h10 = w[d][0]
                    nc.vector.tensor_scalar(
                        out=h10[:], in0=u2_t[:], scalar1=-2.0, scalar2=None,
                        op0=OP.mult,
                    )
                    nc.vector.tensor_add(out=h10[:], in0=h10[:], in1=u_t[:])
                    nc.vector.tensor_add(out=h10[:], in0=h10[:], in1=u3_t[:])
                    h11 = w[d][3]
                    nc.vector.tensor_sub(out=h11[:], in0=u3_t[:], in1=u2_t[:])
                    p1 = w[d][0]
                    nc.vector.tensor_mul(out=p1[:], in0=h10[:], in1=a_t[:])
                    p2 = w[d][3]
                    nc.vector.tensor_mul(out=p2[:], in0=h11[:], in1=b_t[:])
                    # w2 = (p1 - h00) + 1 ; w1 = h00 - p2 ; w0 = -p1 ; w3 = p2
                    nc.vector.tensor_sub(out=w[d][2][:], in0=p1[:], in1=h00[:])
                    nc.vector.tensor_scalar(
                        out=w[d][2][:], in0=w[d][2][:], scalar1=1.0, scalar2=None,
                        op0=OP.add,
                    )
                    nc.vector.tensor_sub(out=w[d][1][:], in0=h00[:], in1=p2[:])
                    nc.vector.tensor_scalar(
                        out=w[d][0][:], in0=p1[:], scalar1=-1.0, scalar2=None,
                        op0=OP.mult,
                    )
                if DEBUG and s == 0:
                    for d in range(D):
                        for c in range(4):
                            nc.sync.dma_start(
                                out=dbg_w[
                                    :, (d * 4 + c) * G : (d * 4 + c + 1) * G
                                ],
                                in_=w[d][c][:],
                            )

                # contraction per tile: layout [i(4)x256, j(4)x64, k(4)x16, v(16)]
                for tl in range(G):
                    t = s * G + tl
                    tview = nbs[tl][:]
                    acc1 = accpool.tile([P, 256], F32, tag="acc1", name="acc1")
                    for k in range(4):
                        kslice = bass.AP(
                            nbs[tl].tensor,
                            tview.offset + k * V,
                            [tview.ap[0], [64, 16], [1, V]],
                        )
                        if k == 0:
                            nc.vector.tensor_scalar(
                                out=acc1[:], in0=kslice,
                                scalar1=w[2][k][:, tl : tl + 1],
                                scalar2=None, op0=OP.mult,
                            )
                        else:
                            nc.vector.scalar_tensor_tensor(
                                out=acc1[:], in0=kslice,
                                scalar=w[2][k][:, tl : tl + 1],
                                in1=acc1[:], op0=OP.mult, op1=OP.add,
                            )
                    acc2 = accpool.tile([P, 64], F32, tag="acc2", name="acc2")
                    for j in range(4):
                        jslice = bass.AP(
                            acc1.tensor,
                            acc1[:].offset + j * V,
                            [acc1[:].ap[0], [64, 4], [1, V]],
                        )
                        if j == 0:
                            nc.vector.tensor_scalar(
                                out=acc2[:], in0=jslice,
                                scalar1=w[1][j][:, tl : tl + 1],
                                scalar2=None, op0=OP.mult,
                            )
                        else:
                            nc.vector.scalar_tensor_tensor(
                                out=acc2[:], in0=jslice,
                                scalar=w[1][j][:, tl : tl + 1],
                                in1=acc2[:], op0=OP.mult, op1=OP.add,
                            )
                    oslice = outbuf[:, t * V : (t + 1) * V]
                    for i in range(4):
                        islice = acc2[:, i * V : (i + 1) * V]
                        if i == 0:
                            nc.vector.tensor_scalar(
                                out=oslice, in0=islice,
                                scalar1=w[0][i][:, tl : tl + 1],
                                scalar2=None, op0=OP.mult,
                            )
                        else:
                            nc.vector.scalar_tensor_tensor(
                                out=oslice, in0=islice,
                                scalar=w[0][i][:, tl : tl + 1],
                                in1=oslice, op0=OP.mult, op1=OP.add,
                            )
            nc.sync.dma_start(out=outb[:], in_=outbuf[:])
    return nc


def _emit_kernel_affine(nc: bass.Bass, k0s, invs):
    """Fast path for affine (uniform-spacing) knots: idx/u/weights are pure
    arithmetic (knot values baked as immediates), rows are exactly 1024 f32
    with layout [ij(16), v(16), k(4)] (k innermost)."""
    xkn = nc.dram_tensor("xkn", [P, D * T + D * N], F32, kind="ExternalInput")
    gtab = nc.dram_tensor("gtab", [NROWS, NB], F32, kind="ExternalInput")
    outb = nc.dram_tensor("outb", [P, T * V], F32, kind="ExternalOutput")

    with tile.TileContext(nc) as tc:
        from contextlib import ExitStack

        with ExitStack() as ctx:
            singles = ctx.enter_context(tc.tile_pool(name="singles", bufs=1))
            nbpool = ctx.enter_context(tc.tile_pool(name="nb", bufs=8))
            accpool = ctx.enter_context(tc.tile_pool(name="acc", bufs=2))
            p1pool = ctx.enter_context(tc.tile_pool(name="p1", bufs=2))

            sb_xk = singles.tile([P, D * T + D * N], F32, tag="sb_xk", name="sb_xk")
            nc.sync.dma_start(out=sb_xk[:], in_=xkn[:])
            sb_x = sb_xk[:, : D * T].rearrange("p (d t) -> p d t", d=D)

            # ---- phase A: idx, u, weights (global, arithmetic) ----
            idx_f = [
                singles.tile([P, T], F32, tag=f"idx{d}", name=f"idx{d}")
                for d in range(D)
            ]
            u_d = [
                singles.tile([P, T], F32, tag=f"u{d}", name=f"u{d}") for d in range(D)
            ]
            scr = [
                singles.tile([P, T], F32, tag=f"sc{i}", name=f"sc{i}") for i in range(4)
            ]
            tA, f_t, u2_t, u3_t = scr
            # wd[d] layout [P, (t,c)]: column t*4+c
            wd = [
                singles.tile([P, T * 4], F32, tag=f"wd{d}", name=f"wd{d}")
                for d in range(D)
            ]
            for d in range(D):
                xd = sb_x[:, d, :]
                nc.vector.tensor_scalar(
                    out=tA[:], in0=xd, scalar1=-float(k0s[d]), scalar2=float(invs[d]),
                    op0=OP.add, op1=OP.mult,
                )
                # floor(t) for t in (-1, 48): round(t - 0.5) via the 2^23
                # magic-number trick (one dual-op instruction)
                nc.vector.tensor_scalar(
                    out=idx_f[d][:], in0=tA[:], scalar1=8388607.5,
                    scalar2=8388608.0, op0=OP.add, op1=OP.subtract,
                )
                nc.vector.tensor_scalar(
                    out=idx_f[d][:], in0=idx_f[d][:], scalar1=float(NI - 1),
                    scalar2=0.0, op0=OP.min, op1=OP.max,
                )
                nc.vector.tensor_sub(out=u_d[d][:], in0=tA[:], in1=idx_f[d][:])

            basef = singles.tile([P, T], F32, tag="basef", name="basef")
            nc.vector.scalar_tensor_tensor(
                out=basef[:], in0=idx_f[1][:], scalar=float(NI), in1=idx_f[2][:],
                op0=OP.mult, op1=OP.add,
            )
            nc.vector.scalar_tensor_tensor(
                out=basef[:], in0=idx_f[0][:], scalar=float(NI * NI), in1=basef[:],
                op0=OP.mult, op1=OP.add,
            )
            gidx = singles.tile([P, T], I32, tag="gidx", name="gidx")
            nc.vector.tensor_copy(out=gidx[:], in_=basef[:])

            for d in range(D):
                u = u_d[d]
                nc.vector.tensor_mul(out=u2_t[:], in0=u[:], in1=u[:])
                nc.vector.tensor_mul(out=u3_t[:], in0=u2_t[:], in1=u[:])
                h00 = idx_f[d]  # reuse (idx no longer needed for this dim)
                nc.vector.tensor_scalar(
                    out=h00[:], in0=u2_t[:], scalar1=-3.0, scalar2=1.0,
                    op0=OP.mult, op1=OP.add,
                )
                nc.vector.scalar_tensor_tensor(
                    out=h00[:], in0=u3_t[:], scalar=2.0, in1=h00[:],
                    op0=OP.mult, op1=OP.add,
                )
                h10 = tA
                nc.vector.tensor_scalar(
                    out=h10[:], in0=u2_t[:], scalar1=-2.0, scalar2=None, op0=OP.mult
                )
                nc.vector.tensor_add(out=h10[:], in0=h10[:], in1=u[:])
                nc.vector.tensor_add(out=h10[:], in0=h10[:], in1=u3_t[:])
                h11 = f_t
                nc.vector.tensor_sub(out=h11[:], in0=u3_t[:], in1=u2_t[:])

                def wcol(c, _d=d):
                    base_ap = wd[_d][:]
                    return bass.AP(wd[_d].tensor, base_ap.offset + c, [base_ap.ap[0], [4, T]])

                # w0 = -0.5*h10 ; w3 = 0.5*h11 ; w1 = h00 - 0.5*h11 ; w2 = 1 - h00 + 0.5*h10
                nc.vector.tensor_scalar(
                    out=wcol(0), in0=h10[:], scalar1=-0.5, scalar2=None, op0=OP.mult
                )
                nc.vector.tensor_scalar(
                    out=wcol(3), in0=h11[:], scalar1=0.5, scalar2=None, op0=OP.mult
                )
                nc.vector.scalar_tensor_tensor(
                    out=wcol(1), in0=h11[:], scalar=-0.5, in1=h00[:],
                    op0=OP.mult, op1=OP.add,
                )
                tmp3 = u2_t
                nc.vector.tensor_scalar(
                    out=tmp3[:], in0=h00[:], scalar1=-1.0, scalar2=1.0,
                    op0=OP.mult, op1=OP.add,
                )
                nc.vector.scalar_tensor_tensor(
                    out=wcol(2), in0=h10[:], scalar=0.5, in1=tmp3[:],
                    op0=OP.mult, op1=OP.add,
                )

            # W12[p, t*16 + i*4 + j] = wd0[t,i] * wd1[t,j]
            w12 = singles.tile([P, T * 16], F32, tag="w12", name="w12")
            for i in range(4):
                nc.vector.tensor_tensor(
                    out=bass.AP(w12.tensor, w12[:].offset + i * 4,
                                [w12[:].ap[0], [16, T], [1, 4]]),
                    in0=bass.AP(wd[1].tensor, wd[1][:].offset,
                                [wd[1][:].ap[0], [4, T], [1, 4]]),
                    in1=bass.AP(wd[0].tensor, wd[0][:].offset + i,
                                [wd[0][:].ap[0], [4, T], [0, 4]]),
                    op=OP.mult,
                )

            # ---- phase B ----
            outbuf = singles.tile([P, T * V], F32, tag="outbuf", name="outbuf")
            for s in range(NST * REPS):
                s = s % NST
                nbs = [
                    nbpool.tile([P, NB], F32, tag="nb", name=f"nb_{s}_{tl}")
                    for tl in range(G)
                ]
                for tl in range(G):
                    t = s * G + tl
                    nc.gpsimd.indirect_dma_start(
                        out=nbs[tl][:],
                        out_offset=None,
                        in_=gtab[:],
                        in_offset=bass.IndirectOffsetOnAxis(
                            ap=gidx[:, t : t + 1], axis=0
                        ),
                    )
                acc = accpool.tile([P, G * 256], F32, tag="acct", name="acct")
                for tl in range(G):
                    t = s * G + tl
                    tv = nbs[tl][:]
                    for k in range(4):
                        kslice = bass.AP(
                            nbs[tl].tensor, tv.offset + k, [tv.ap[0], [4, 256]]
                        )
                        oslice = acc[:, tl * 256 : (tl + 1) * 256]
                        sc = bass.AP(
                            wd[2].tensor, wd[2][:].offset + t * 4 + k,
                            [wd[2][:].ap[0], [1, 1]],
                        )
                        if k == 0:
                            nc.vector.tensor_scalar(
                                out=oslice, in0=kslice, scalar1=sc, scalar2=None,
                                op0=OP.mult,
                            )
                        else:
                            nc.vector.scalar_tensor_tensor(
                                out=oslice, in0=kslice, scalar=sc, in1=oslice,
                                op0=OP.mult, op1=OP.add,
                            )
                # stage 2 (batched over supertile):
                # P1v[t, v, ij] = acc[t, ij, v] * W12[t, ij]
                p1v = p1pool.tile([P, G * 256], F32, tag="p1v", name="p1v")
                nc.vector.tensor_tensor(
                    out=bass.AP(p1v.tensor, p1v[:].offset,
                                [p1v[:].ap[0], [256, G], [16, 16], [1, 16]]),
                    in0=bass.AP(acc.tensor, acc[:].offset,
                                [acc[:].ap[0], [256, G], [1, 16], [16, 16]]),
                    in1=bass.AP(w12.tensor, w12[:].offset + s * G * 16,
                                [w12[:].ap[0], [16, G], [0, 16], [1, 16]]),
                    op=OP.mult,
                )
                # note: iteration dims above are (t, v, ij) for out;
                # in0 iterates (t, v, ij) -> acc[t, ij, v] via strides (256,1,16)
                nc.vector.tensor_reduce(
                    out=outbuf[:, s * G * V : (s + 1) * G * V],
                    in_=bass.AP(p1v.tensor, p1v[:].offset,
                                [p1v[:].ap[0], [16, G * 16], [1, 16]]),
                    axis=mybir.AxisListType.X,
                    op=OP.add,
                )
            nc.sync.dma_start(out=outb[:], in_=outbuf[:])
    return nc


_NC_CACHE = {}


def _affine_params(knots):
    """Return (k0s, invs) if each dim's knots are (near-)affine, else None."""
    kn = np.asarray(knots, dtype=np.float64)
    k0s, invs = [], []
    for d in range(D):
        kd = kn[d]
        step = (kd[-1] - kd[0]) / (N - 1)
        if step <= 0:
            return None
        fit = kd[0] + step * np.arange(N)
        if not np.allclose(kd, fit, rtol=0, atol=1e-6 * max(1.0, abs(step) * N)):
            return None
        k0s.append(float(kd[0]))
        invs.append(float(1.0 / step))
    return k0s, invs


def _get_nc(mode="general", params=None):
    key = (mode, tuple(params[0]) + tuple(params[1]) if params else None, REPS)
    if key not in _NC_CACHE:
        nc = bacc.Bacc(None, target_bir_lowering=False)
        if mode == "affine":
            _emit_kernel_affine(nc, params[0], params[1])
        else:
            _emit_kernel_general(nc)
        nc.compile()
        _NC_CACHE[key] = nc
    return _NC_CACHE[key]


_PREP_CACHE = {}


def _host_prep(knots, grid, affine=False):
    key = (id(knots), id(grid), affine)
    if key in _PREP_CACHE:
        return _PREP_CACHE[key]
    from numpy.lib.stride_tricks import sliding_window_view

    gp = np.asarray(grid, dtype=np.float32)
    for ax in range(3):
        lo = 2.0 * np.take(gp, 0, axis=ax) - np.take(gp, 1, axis=ax)
        hi = 2.0 * np.take(gp, -1, axis=ax) - np.take(gp, -2, axis=ax)
        gp = np.concatenate(
            [np.expand_dims(lo, ax), gp, np.expand_dims(hi, ax)], axis=ax
        )
    sw = sliding_window_view(gp, (4, 4, 4), axis=(0, 1, 2))
    if affine:
        # rows [47,47,47, (ij)16, v16, k4] = 1024 f32 (4KB), k innermost
        nbr = np.ascontiguousarray(
            sw.transpose(0, 1, 2, 4, 5, 3, 6)  # [.., i, j, v, k]
        ).reshape(NROWS, NB)
        kn = np.asarray(knots, dtype=np.float32)
        knr = np.ascontiguousarray(
            np.broadcast_to(kn.reshape(1, D * N), (P, D * N))
        ).astype(np.float32)
        _PREP_CACHE[key] = (nbr, knr)
        return nbr, knr
    # general path: rows [.., i, j, k, v] + embedded knot windows
    nbr = sw.transpose(0, 1, 2, 4, 5, 6, 3).reshape(NROWS, NB)
    # knot windows per dim: ktw[d][i] = padded_knots[d][i:i+4]
    kn = np.asarray(knots, dtype=np.float32)
    tp = np.concatenate(
        [2.0 * kn[:, :1] - kn[:, 1:2], kn, 2.0 * kn[:, -1:] - kn[:, -2:-1]], axis=1
    )
    ktw = sliding_window_view(tp, 4, axis=1)  # [3, 47, 4]
    gtab = np.empty((NROWS, ROW), dtype=np.float32)
    gtab[:, :NB] = nbr
    kv = gtab[:, NB : NB + 12].reshape(NI, NI, NI, 3, 4)
    kv[:, :, :, 0, :] = ktw[0][:, None, None, :]
    kv[:, :, :, 1, :] = ktw[1][None, :, None, :]
    kv[:, :, :, 2, :] = ktw[2][None, None, :, :]
    gtab[:, NB + 12 :] = 0.0
    knr = np.ascontiguousarray(
        np.broadcast_to(kn.reshape(1, D * N), (P, D * N))
    ).astype(np.float32)
    _PREP_CACHE[key] = (gtab, knr)
    return gtab, knr


def kernel(x, knots, grid):
    x = np.asarray(x, dtype=np.float32)
    params = _affine_params(knots)
    if params is not None:
        gtab, knr = _host_prep(knots, grid, affine=True)
        nc = _get_nc("affine", params)
    else:
        gtab, knr = _host_prep(knots, grid)
        nc = _get_nc()
    in_maps = []
    for c in range(NCORES):
        xs = x[c * BS : (c + 1) * BS]  # [BS, 3]
        xTc = np.ascontiguousarray(
            xs.reshape(T, P, D).transpose(1, 2, 0).reshape(P, D * T)
        )  # [p, d*T + t] = x[t*P + p, d]
        xknc = np.concatenate([xTc, knr], axis=1)
        in_maps.append({"xkn": xknc, "gtab": gtab})
    res = run_bass_kernel_spmd(nc, in_maps, core_ids=list(range(NCORES)))
    outs = []
    for c in range(NCORES):
        ob = res.results[c]["outb"].reshape(P, T, V)  # [p, t, v]
        outs.append(np.ascontiguousarray(ob.transpose(1, 0, 2)).reshape(BS, V))
    return np.concatenate(outs, axis=0)


# revision 15
# speedup vs baseline: 1.2912x; 1.0367x over previous
"""Trainium2 Bass kernel for non-uniform 3D Catmull-Rom spline interpolation.

Problem: x [131072, 3] query points, knots [3, 48], grid [48,48,48,16]
-> out [131072, 16].

Strategy (data-parallel over the batch across 8 cores):
  Host: pad the grid to [50,50,50,16]; build a replicated gather table
    gtab[(p0, q0, r0), 1040] whose 4160B rows hold the full 4x4x4x16
    neighborhood gp[p0:p0+4, q0:q0+4, r0:r0+4, :] (1024 f32) plus the 12
    knot-window values [tm1,t0,t1,t2] per dim (keyed by p0/q0/r0), padded
    to 1040 f32.  A query's entire working set is then ONE contiguous 4KB+
    row -> one [128,1]-index indirect DMA per 128-query tile (the only
    reliably-ordered indirect-DMA shape on TRN2 SWDGE; >=512B descriptors
    run at full DMA rate).
  Device (per core, 16384 queries = 128 tiles of 128 queries on partitions):
    - searchsorted per dim via 47 fused (x >= knot_j) + acc ops (batched
      over all queries)
    - row index = idx0*47*47 + idx1*47 + idx2 -> gidx [128,128] int32
    - per supertile of G tiles: G gathers, Catmull-Rom weights from the
      embedded knot windows (batched [128,G] DVE ops), then per-tile
      contraction k -> j -> i with tensor_scalar / scalar_tensor_tensor
      fused multiply-adds using per-partition scalars (per-query weights)
"""

import numpy as np

import concourse.bass as bass
import concourse.bacc as bacc
import concourse.tile as tile
from concourse import mybir
from concourse.bass_utils import run_bass_kernel_spmd

# ---- problem constants (hardcoded per harness contract) ----
B, D, N, V = 131072, 3, 48, 16
NCORES = 8
BS = B // NCORES  # 16384 queries per core
P = 128           # partitions
T = BS // P       # 128 tiles of 128 queries
NP = N + 2        # 50 (padded grid extent)
NI = N - 1        # 47 interval starts (idx in [0, 46])
NB = 4 * 4 * 4 * V       # 1024 f32 neighborhood per query
ROW = NB + 16            # + [3 dims x 4 knot vals] + 4 pad = 1040 f32 (4160B)
NROWS = NI * NI * NI     # 103823 gather-table rows
G = 16            # tiles per supertile (weight-batching granularity)
NST = T // G      # supertiles

F32 = mybir.dt.float32
I32 = mybir.dt.int32
OP = mybir.AluOpType

DEBUG = False
REPS = 1  # phase-B repetitions (differential HW timing; harness uses 1)


def _emit_kernel_general(nc: bass.Bass):
    xkn = nc.dram_tensor("xkn", [P, D * T + D * N], F32, kind="ExternalInput")
    gtab = nc.dram_tensor("gtab", [NROWS, ROW], F32, kind="ExternalInput")
    outb = nc.dram_tensor("outb", [P, T * V], F32, kind="ExternalOutput")
    if DEBUG:
        dbg_gidx = nc.dram_tensor("dbg_gidx", [P, T], I32, kind="ExternalOutput")
        dbg_nb = nc.dram_tensor("dbg_nb", [P, G * ROW], F32, kind="ExternalOutput")
        dbg_w = nc.dram_tensor("dbg_w", [P, D * 4 * G], F32, kind="ExternalOutput")

    with tile.TileContext(nc) as tc:
        from contextlib import ExitStack

        with ExitStack() as ctx:
            singles = ctx.enter_context(tc.tile_pool(name="singles", bufs=1))
            nbpool = ctx.enter_context(tc.tile_pool(name="nb", bufs=2 * G))
            wpool = ctx.enter_context(tc.tile_pool(name="wp", bufs=2))
            accpool = ctx.enter_context(tc.tile_pool(name="acc", bufs=4))

            # ---------------- load inputs (one DMA) ----------------
            sb_xk = singles.tile([P, D * T + D * N], F32, tag="sb_xk", name="sb_xk")
            nc.sync.dma_start(out=sb_xk[:], in_=xkn[:])
            sb_x = sb_xk[:, : D * T].rearrange("p (d t) -> p d t", d=D)
            sb_kn = sb_xk[:, D * T :].rearrange("p (d n) -> p d n", d=D)

            # ---------------- phase A: searchsorted + row index ----------------
            idx_f = [
                singles.tile([P, T], F32, tag=f"idx{d}", name=f"idx{d}")
                for d in range(D)
            ]
            for d in range(D):
                xd = sb_x[:, d, :]
                acc = idx_f[d]
                nc.vector.tensor_scalar(
                    out=acc[:], in0=xd, scalar1=sb_kn[:, d, 1:2], scalar2=None,
                    op0=OP.is_ge,
                )
                for j in range(2, N):
                    nc.vector.scalar_tensor_tensor(
                        out=acc[:], in0=xd, scalar=sb_kn[:, d, j : j + 1],
                        in1=acc[:], op0=OP.is_ge, op1=OP.add,
                    )
                nc.vector.tensor_scalar(
                    out=acc[:], in0=acc[:], scalar1=float(NI - 1), scalar2=None,
                    op0=OP.min,
                )

            # base row = idx0*NI*NI + idx1*NI + idx2
            basef = singles.tile([P, T], F32, tag="basef", name="basef")
            nc.vector.scalar_tensor_tensor(
                out=basef[:], in0=idx_f[1][:], scalar=float(NI), in1=idx_f[2][:],
                op0=OP.mult, op1=OP.add,
            )
            nc.vector.scalar_tensor_tensor(
                out=basef[:], in0=idx_f[0][:], scalar=float(NI * NI), in1=basef[:],
                op0=OP.mult, op1=OP.add,
            )
            gidx = singles.tile([P, T], I32, tag="gidx", name="gidx")
            nc.vector.tensor_copy(out=gidx[:], in_=basef[:])
            if DEBUG:
                nc.sync.dma_start(out=dbg_gidx[:], in_=gidx[:])

            # ---------------- phase B: gather + weights + contract ----------
            outbuf = singles.tile([P, T * V], F32, tag="outbuf", name="outbuf")
            for s in range(NST * REPS):
                s = s % NST
                nbs = [
                    nbpool.tile([P, ROW], F32, tag="nb", name=f"nb_{s}_{tl}")
                    for tl in range(G)
                ]
                for tl in range(G):
                    t = s * G + tl
                    nc.gpsimd.indirect_dma_start(
                        out=nbs[tl][:],
                        out_offset=None,
                        in_=gtab[:],
                        in_offset=bass.IndirectOffsetOnAxis(
                            ap=gidx[:, t : t + 1], axis=0
                        ),
                    )
                if DEBUG and s == 0:
                    for tl in range(G):
                        nc.sync.dma_start(
                            out=dbg_nb[:, tl * ROW : (tl + 1) * ROW], in_=nbs[tl][:]
                        )

                # collect the embedded knot windows: kvst[p, tl*16 + (d*4+c)]
                kvst = wpool.tile([P, G * 16], F32, tag="kvst", name="kvst")
                for tl in range(G):
                    nc.vector.tensor_copy(
                        out=kvst[:, tl * 16 : (tl + 1) * 16], in_=nbs[tl][:, NB:]
                    )

                # weights per dim, batched over the supertile: [P, G] ops
                w = [
                    [
                        wpool.tile([P, G], F32, tag=f"w{d}{c}", name=f"w{d}{c}")
                        for c in range(4)
                    ]
                    for d in range(D)
                ]
                scr = [
                    wpool.tile([P, G], F32, tag=f"scr{i}", name=f"scr{i}")
                    for i in range(6)
                ]
                u_t, u2_t, u3_t, s0, s1, s2 = scr
                for d in range(D):
                    xd = sb_x[:, d, s * G : (s + 1) * G]
                    kbase = kvst[:]

                    def kvv(c, _kb=kbase, _d=d):
                        return bass.AP(
                            kvst.tensor,
                            _kb.offset + _d * 4 + c,
                            [_kb.ap[0], [16, G]],
                        )

                    tm1, t0, t1, t2 = kvv(0), kvv(1), kvv(2), kvv(3)
                    dt = s0
                    nc.vector.tensor_sub(out=dt[:], in0=t1, in1=t0)
                    xm = s1
                    nc.vector.tensor_sub(out=xm[:], in0=xd, in1=t0)  # x - t0
                    rcp = s2
                    nc.vector.reciprocal(out=rcp[:], in_=dt[:])
                    nc.vector.tensor_mul(out=u_t[:], in0=xm[:], in1=rcp[:])  # u
                    nc.vector.tensor_mul(out=u2_t[:], in0=u_t[:], in1=u_t[:])
                    nc.vector.tensor_mul(out=u3_t[:], in0=u2_t[:], in1=u_t[:])
                    a_t = s1
                    nc.vector.tensor_sub(out=a_t[:], in0=t1, in1=tm1)
                    nc.vector.reciprocal(out=a_t[:], in_=a_t[:])
                    nc.vector.tensor_mul(out=a_t[:], in0=a_t[:], in1=dt[:])
                    b_t = s2
                    nc.vector.tensor_sub(out=b_t[:], in0=t2, in1=t0)
                    nc.vector.reciprocal(out=b_t[:], in_=b_t[:])
                    nc.vector.tensor_mul(out=b_t[:], in0=b_t[:], in1=dt[:])
                    # h00 = 2u3 - 3u2 + 1 ; h10 = u3 - 2u2 + u ; h11 = u3 - u2
                    h00 = w[d][1]
                    nc.vector.tensor_scalar(
                        out=h00[:], in0=u2_t[:], scalar1=-3.0, scalar2=1.0,
                        op0=OP.mult, op1=OP.add,
                    )
                    nc.vector.scalar_tensor_tensor(
                        out=h00[:], in0=u3_t[:], scalar=2.0, in1=h00[:],
                        op0=OP.mult, op1=OP.add,
                    )
                    h10 = w[d][0]
                    nc.vector.tensor_scalar(
                        out=h10[:], in0=u2_t[:], scalar1=-2.0, scalar2=None,
                        op0=OP.mult,
                    )
                    nc.vector.tensor_add(out=h10[:], in0=h10[:], in1=u_t[:])
                    nc.vector.tensor_add(out=h10[:], in0=h10[:], in1=u3_t[:])
                    h11 = w[d][3]
                    nc.vector.tensor_sub(out=h11[:], in0=u3_t[:], in1=u2_t[:])
                    p1 = w[d][0]
                    nc.vector.tensor_mul(out=p1[:], in0=h10[:], in1=a_t[:])
                    p2 = w[d][3]
                    nc.vector.tensor_mul(out=p2[:], in0=h11[:], in1=b_t[:])
                    # w2 = (p1 - h00) + 1 ; w1 = h00 - p2 ; w0 = -p1 ; w3 = p2
                    nc.vector.tensor_sub(out=w[d][2][:], in0=p1[:], in1=h00[:])
                    nc.vector.tensor_scalar(
                        out=w[d][2][:], in0=w[d][2][:], scalar1=1.0, scalar2=None,
                        op0=OP.add,
                    )
                    nc.vector.tensor_sub(out=w[d][1][:], in0=h00[:], in1=p2[:])
                    nc.vector.tensor_scalar(
                        out=w[d][0][:], in0=p1[:], scalar1=-1.0, scalar2=None,
                        op0=OP.mult,
                    )
                if DEBUG and s == 0:
                    for d in range(D):
                        for c in range(4):
                            nc.sync.dma_start(
                                out=dbg_w[
                                    :, (d * 4 + c) * G : (d * 4 + c + 1) * G
                                ],
                                in_=w[d][c][:],
                            )

                # contraction per tile: layout [i(4)x256, j(4)x64, k(4)x16, v(16)]
                for tl in range(G):
                    t = s * G + tl
                    tview = nbs[tl][:]
                    acc1 = accpool.tile([P, 256], F32, tag="acc1", name="acc1")
                    for k in range(4):
                        kslice = bass.AP(
                            nbs[tl].tensor,
                            tview.offset + k * V,
                            [tview.ap[0], [64, 16], [1, V]],
                        )
                        if k == 0:
                            nc.vector.tensor_scalar(
                                out=acc1[:], in0=kslice,
                                scalar1=w[2][k][:, tl : tl + 1],
                                scalar2=None, op0=OP.mult,
                            )
                        else:
                            nc.vector.scalar_tensor_tensor(
                                out=acc1[:], in0=kslice,
                                scalar=w[2][k][:, tl : tl + 1],
                                in1=acc1[:], op0=OP.mult, op1=OP.add,
                            )
                    acc2 = accpool.tile([P, 64], F32, tag="acc2", name="acc2")
                    for j in range(4):
                        jslice = bass.AP(
                            acc1.tensor,
                            acc1[:].offset + j * V,
                            [acc1[:].ap[0], [64, 4], [1, V]],
                        )
                        if j == 0:
                            nc.vector.tensor_scalar(
                                out=acc2[:], in0=jslice,
                                scalar1=w[1][j][:, tl : tl + 1],
                                scalar2=None, op0=OP.mult,
                            )
                        else:
                            nc.vector.scalar_tensor_tensor(
                                out=acc2[:], in0=jslice,
                                scalar=w[1][j][:, tl : tl + 1],
                                in1=acc2[:], op0=OP.mult, op1=OP.add,
                            )
                    oslice = outbuf[:, t * V : (t + 1) * V]
                    for i in range(4):
                        islice = acc2[:, i * V : (i + 1) * V]
                        if i == 0:
                            nc.vector.tensor_scalar(
                                out=oslice, in0=islice,
                                scalar1=w[0][i][:, tl : tl + 1],
                                scalar2=None, op0=OP.mult,
                            )
                        else:
                            nc.vector.scalar_tensor_tensor(
                                out=oslice, in0=islice,
                                scalar=w[0][i][:, tl : tl + 1],
                                in1=oslice, op0=OP.mult, op1=OP.add,
                            )
            nc.sync.dma_start(out=outb[:], in_=outbuf[:])
    return nc


def _emit_kernel_affine(nc: bass.Bass, k0s, invs):
    """Fast path for affine (uniform-spacing) knots: idx/u/weights are pure
    arithmetic (knot values baked as immediates), rows are exactly 1024 f32
    with layout [ij(16), v(16), k(4)] (k innermost)."""
    xkn = nc.dram_tensor("xkn", [P, D * T + D * N], F32, kind="ExternalInput")
    gtab = nc.dram_tensor("gtab", [NROWS, NB], F32, kind="ExternalInput")
    outb = nc.dram_tensor("outb", [P, T * V], F32, kind="ExternalOutput")

    with tile.TileContext(nc) as tc:
        from contextlib import ExitStack

        with ExitStack() as ctx:
            singles = ctx.enter_context(tc.tile_pool(name="singles", bufs=1))
            nbpool = ctx.enter_context(tc.tile_pool(name="nb", bufs=8))
            accpool = ctx.enter_context(tc.tile_pool(name="acc", bufs=2))
            p1pool = ctx.enter_context(tc.tile_pool(name="p1", bufs=2))

            sb_xk = singles.tile([P, D * T + D * N], F32, tag="sb_xk", name="sb_xk")
            nc.sync.dma_start(out=sb_xk[:], in_=xkn[:])
            sb_x = sb_xk[:, : D * T].rearrange("p (d t) -> p d t", d=D)

            # ---- phase A: idx, u, weights (global, arithmetic) ----
            idx_f = [
                singles.tile([P, T], F32, tag=f"idx{d}", name=f"idx{d}")
                for d in range(D)
            ]
            u_d = [
                singles.tile([P, T], F32, tag=f"u{d}", name=f"u{d}") for d in range(D)
            ]
            scr = [
                singles.tile([P, T], F32, tag=f"sc{i}", name=f"sc{i}") for i in range(4)
            ]
            tA, f_t, u2_t, u3_t = scr
            # wd[d] layout [P, (t,c)]: column t*4+c
            wd = [
                singles.tile([P, T * 4], F32, tag=f"wd{d}", name=f"wd{d}")
                for d in range(D)
            ]
            for d in range(D):
                xd = sb_x[:, d, :]
                nc.vector.tensor_scalar(
                    out=tA[:], in0=xd, scalar1=-float(k0s[d]), scalar2=float(invs[d]),
                    op0=OP.add, op1=OP.mult,
                )
                # floor(t) for t in (-1, 48): round(t - 0.5) via the 2^23
                # magic-number trick (one dual-op instruction)
                nc.vector.tensor_scalar(
                    out=idx_f[d][:], in0=tA[:], scalar1=8388607.5,
                    scalar2=8388608.0, op0=OP.add, op1=OP.subtract,
                )
                nc.vector.tensor_scalar(
                    out=idx_f[d][:], in0=idx_f[d][:], scalar1=float(NI - 1),
                    scalar2=0.0, op0=OP.min, op1=OP.max,
                )
                nc.vector.tensor_sub(out=u_d[d][:], in0=tA[:], in1=idx_f[d][:])

            basef = singles.tile([P, T], F32, tag="basef", name="basef")
            nc.vector.scalar_tensor_tensor(
                out=basef[:], in0=idx_f[1][:], scalar=float(NI), in1=idx_f[2][:],
                op0=OP.mult, op1=OP.add,
            )
            nc.vector.scalar_tensor_tensor(
                out=basef[:], in0=idx_f[0][:], scalar=float(NI * NI), in1=basef[:],
                op0=OP.mult, op1=OP.add,
            )
            gidx = singles.tile([P, T], I32, tag="gidx", name="gidx")
            nc.vector.tensor_copy(out=gidx[:], in_=basef[:])

            for d in range(D):
                u = u_d[d]
                nc.vector.tensor_mul(out=u2_t[:], in0=u[:], in1=u[:])
                nc.vector.tensor_mul(out=u3_t[:], in0=u2_t[:], in1=u[:])
                h00 = idx_f[d]  # reuse (idx no longer needed for this dim)
                nc.vector.tensor_scalar(
                    out=h00[:], in0=u2_t[:], scalar1=-3.0, scalar2=1.0,
                    op0=OP.mult, op1=OP.add,
                )
                nc.vector.scalar_tensor_tensor(
                    out=h00[:], in0=u3_t[:], scalar=2.0, in1=h00[:],
                    op0=OP.mult, op1=OP.add,
                )
                h10 = tA
                nc.vector.tensor_scalar(
                    out=h10[:], in0=u2_t[:], scalar1=-2.0, scalar2=None, op0=OP.mult
                )
                nc.vector.tensor_add(out=h10[:], in0=h10[:], in1=u[:])
                nc.vector.tensor_add(out=h10[:], in0=h10[:], in1=u3_t[:])
                h11 = f_t
                nc.vector.tensor_sub(out=h11[:], in0=u3_t[:], in1=u2_t[:])

                def wcol(c, _d=d):
                    base_ap = wd[_d][:]
                    return bass.AP(wd[_d].tensor, base_ap.offset + c, [base_ap.ap[0], [4, T]])

                # w0 = -0.5*h10 ; w3 = 0.5*h11 ; w1 = h00 - 0.5*h11 ; w2 = 1 - h00 + 0.5*h10
                nc.vector.tensor_scalar(
                    out=wcol(0), in0=h10[:], scalar1=-0.5, scalar2=None, op0=OP.mult
                )
                nc.vector.tensor_scalar(
                    out=wcol(3), in0=h11[:], scalar1=0.5, scalar2=None, op0=OP.mult
                )
                nc.vector.scalar_tensor_tensor(
                    out=wcol(1), in0=h11[:], scalar=-0.5, in1=h00[:],
                    op0=OP.mult, op1=OP.add,
                )
                tmp3 = u2_t
                nc.vector.tensor_scalar(
                    out=tmp3[:], in0=h00[:], scalar1=-1.0, scalar2=1.0,
                    op0=OP.mult, op1=OP.add,
                )
                nc.vector.scalar_tensor_tensor(
                    out=wcol(2), in0=h10[:], scalar=0.5, in1=tmp3[:],
                    op0=OP.mult, op1=OP.add,
                )

            # W12[p, t*16 + i*4 + j] = wd0[t,i] * wd1[t,j]
            w12 = singles.tile([P, T * 16], F32, tag="w12", name="w12")
            for i in range(4):
                nc.vector.tensor_tensor(
                    out=bass.AP(w12.tensor, w12[:].offset + i * 4,
                                [w12[:].ap[0], [16, T], [1, 4]]),
                    in0=bass.AP(wd[1].tensor, wd[1][:].offset,
                                [wd[1][:].ap[0], [4, T], [1, 4]]),
                    in1=bass.AP(wd[0].tensor, wd[0][:].offset + i,
                                [wd[0][:].ap[0], [4, T], [0, 4]]),
                    op=OP.mult,
                )

            # ---- phase B ----
            outbuf = singles.tile([P, T * V], F32, tag="outbuf", name="outbuf")
            for s in range(NST * REPS):
                s = s % NST
                nbs = [
                    nbpool.tile([P, NB], F32, tag="nb", name=f"nb_{s}_{tl}")
                    for tl in range(G)
                ]
                for tl in range(G):
                    t = s * G + tl
                    nc.gpsimd.indirect_dma_start(
                        out=nbs[tl][:],
                        out_offset=None,
                        in_=gtab[:],
                        in_offset=bass.IndirectOffsetOnAxis(
                            ap=gidx[:, t : t + 1], axis=0
                        ),
                    )
                acc = accpool.tile([P, G * 256], F32, tag="acct", name="acct")
                for tl in range(G):
                    t = s * G + tl
                    tv = nbs[tl][:]
                    for k in range(4):
                        kslice = bass.AP(
                            nbs[tl].tensor, tv.offset + k * 256, [tv.ap[0], [1, 256]]
                        )
                        oslice = acc[:, tl * 256 : (tl + 1) * 256]
                        sc = bass.AP(
                            wd[2].tensor, wd[2][:].offset + t * 4 + k,
                            [wd[2][:].ap[0], [1, 1]],
                        )
                        if k == 0:
                            nc.vector.tensor_scalar(
                                out=oslice, in0=kslice, scalar1=sc, scalar2=None,
                                op0=OP.mult,
                            )
                        else:
                            nc.vector.scalar_tensor_tensor(
                                out=oslice, in0=kslice, scalar=sc, in1=oslice,
                                op0=OP.mult, op1=OP.add,
                            )
                # stage 2 (batched over supertile):
                # P1v[t, v, ij] = acc[t, ij, v] * W12[t, ij]
                p1v = p1pool.tile([P, G * 256], F32, tag="p1v", name="p1v")
                nc.vector.tensor_tensor(
                    out=bass.AP(p1v.tensor, p1v[:].offset,
                                [p1v[:].ap[0], [256, G], [16, 16], [1, 16]]),
                    in0=bass.AP(acc.tensor, acc[:].offset,
                                [acc[:].ap[0], [256, G], [1, 16], [16, 16]]),
                    in1=bass.AP(w12.tensor, w12[:].offset + s * G * 16,
                                [w12[:].ap[0], [16, G], [0, 16], [1, 16]]),
                    op=OP.mult,
                )
                # note: iteration dims above are (t, v, ij) for out;
                # in0 iterates (t, v, ij) -> acc[t, ij, v] via strides (256,1,16)
                nc.vector.tensor_reduce(
                    out=outbuf[:, s * G * V : (s + 1) * G * V],
                    in_=bass.AP(p1v.tensor, p1v[:].offset,
                                [p1v[:].ap[0], [16, G * 16], [1, 16]]),
                    axis=mybir.AxisListType.X,
                    op=OP.add,
                )
            nc.sync.dma_start(out=outb[:], in_=outbuf[:])
    return nc


_NC_CACHE = {}


def _affine_params(knots):
    """Return (k0s, invs) if each dim's knots are (near-)affine, else None."""
    kn = np.asarray(knots, dtype=np.float64)
    k0s, invs = [], []
    for d in range(D):
        kd = kn[d]
        step = (kd[-1] - kd[0]) / (N - 1)
        if step <= 0:
            return None
        fit = kd[0] + step * np.arange(N)
        if not np.allclose(kd, fit, rtol=0, atol=1e-6 * max(1.0, abs(step) * N)):
            return None
        k0s.append(float(kd[0]))
        invs.append(float(1.0 / step))
    return k0s, invs


def _get_nc(mode="general", params=None):
    key = (mode, tuple(params[0]) + tuple(params[1]) if params else None, REPS)
    if key not in _NC_CACHE:
        nc = bacc.Bacc(None, target_bir_lowering=False)
        if mode == "affine":
            _emit_kernel_affine(nc, params[0], params[1])
        else:
            _emit_kernel_general(nc)
        nc.compile()
        _NC_CACHE[key] = nc
    return _NC_CACHE[key]


_PREP_CACHE = {}


def _host_prep(knots, grid, affine=False):
    key = (id(knots), id(grid), affine)
    if key in _PREP_CACHE:
        return _PREP_CACHE[key]
    from numpy.lib.stride_tricks import sliding_window_view

    gp = np.asarray(grid, dtype=np.float32)
    for ax in range(3):
        lo = 2.0 * np.take(gp, 0, axis=ax) - np.take(gp, 1, axis=ax)
        hi = 2.0 * np.take(gp, -1, axis=ax) - np.take(gp, -2, axis=ax)
        gp = np.concatenate(
            [np.expand_dims(lo, ax), gp, np.expand_dims(hi, ax)], axis=ax
        )
    sw = sliding_window_view(gp, (4, 4, 4), axis=(0, 1, 2))
    if affine:
        # rows [47,47,47, k4, (ij)16, v16] = 1024 f32 (4KB): per-k contiguous
        nbr = np.ascontiguousarray(
            sw.transpose(0, 1, 2, 6, 4, 5, 3)  # [.., k, i, j, v]
        ).reshape(NROWS, NB)
        kn = np.asarray(knots, dtype=np.float32)
        knr = np.ascontiguousarray(
            np.broadcast_to(kn.reshape(1, D * N), (P, D * N))
        ).astype(np.float32)
        _PREP_CACHE[key] = (nbr, knr)
        return nbr, knr
    # general path: rows [.., i, j, k, v] + embedded knot windows
    nbr = sw.transpose(0, 1, 2, 4, 5, 6, 3).reshape(NROWS, NB)
    # knot windows per dim: ktw[d][i] = padded_knots[d][i:i+4]
    kn = np.asarray(knots, dtype=np.float32)
    tp = np.concatenate(
        [2.0 * kn[:, :1] - kn[:, 1:2], kn, 2.0 * kn[:, -1:] - kn[:, -2:-1]], axis=1
    )
    ktw = sliding_window_view(tp, 4, axis=1)  # [3, 47, 4]
    gtab = np.empty((NROWS, ROW), dtype=np.float32)
    gtab[:, :NB] = nbr
    kv = gtab[:, NB : NB + 12].reshape(NI, NI, NI, 3, 4)
    kv[:, :, :, 0, :] = ktw[0][:, None, None, :]
    kv[:, :, :, 1, :] = ktw[1][None, :, None, :]
    kv[:, :, :, 2, :] = ktw[2][None, None, :, :]
    gtab[:, NB + 12 :] = 0.0
    knr = np.ascontiguousarray(
        np.broadcast_to(kn.reshape(1, D * N), (P, D * N))
    ).astype(np.float32)
    _PREP_CACHE[key] = (gtab, knr)
    return gtab, knr


def kernel(x, knots, grid):
    x = np.asarray(x, dtype=np.float32)
    params = _affine_params(knots)
    if params is not None:
        gtab, knr = _host_prep(knots, grid, affine=True)
        nc = _get_nc("affine", params)
    else:
        gtab, knr = _host_prep(knots, grid)
        nc = _get_nc()
    in_maps = []
    for c in range(NCORES):
        xs = x[c * BS : (c + 1) * BS]  # [BS, 3]
        xTc = np.ascontiguousarray(
            xs.reshape(T, P, D).transpose(1, 2, 0).reshape(P, D * T)
        )  # [p, d*T + t] = x[t*P + p, d]
        xknc = np.concatenate([xTc, knr], axis=1)
        in_maps.append({"xkn": xknc, "gtab": gtab})
    res = run_bass_kernel_spmd(nc, in_maps, core_ids=list(range(NCORES)))
    outs = []
    for c in range(NCORES):
        ob = res.results[c]["outb"].reshape(P, T, V)  # [p, t, v]
        outs.append(np.ascontiguousarray(ob.transpose(1, 0, 2)).reshape(BS, V))
    return np.concatenate(outs, axis=0)


# revision 16
# speedup vs baseline: 1.3157x; 1.0190x over previous
"""Trainium2 Bass kernel for non-uniform 3D Catmull-Rom spline interpolation.

Problem: x [131072, 3] query points, knots [3, 48], grid [48,48,48,16]
-> out [131072, 16].

Strategy (data-parallel over the batch across 8 cores):
  Host: pad the grid to [50,50,50,16]; build a replicated gather table
    gtab[(p0, q0, r0), 1040] whose 4160B rows hold the full 4x4x4x16
    neighborhood gp[p0:p0+4, q0:q0+4, r0:r0+4, :] (1024 f32) plus the 12
    knot-window values [tm1,t0,t1,t2] per dim (keyed by p0/q0/r0), padded
    to 1040 f32.  A query's entire working set is then ONE contiguous 4KB+
    row -> one [128,1]-index indirect DMA per 128-query tile (the only
    reliably-ordered indirect-DMA shape on TRN2 SWDGE; >=512B descriptors
    run at full DMA rate).
  Device (per core, 16384 queries = 128 tiles of 128 queries on partitions):
    - searchsorted per dim via 47 fused (x >= knot_j) + acc ops (batched
      over all queries)
    - row index = idx0*47*47 + idx1*47 + idx2 -> gidx [128,128] int32
    - per supertile of G tiles: G gathers, Catmull-Rom weights from the
      embedded knot windows (batched [128,G] DVE ops), then per-tile
      contraction k -> j -> i with tensor_scalar / scalar_tensor_tensor
      fused multiply-adds using per-partition scalars (per-query weights)
"""

import numpy as np

import concourse.bass as bass
import concourse.bacc as bacc
import concourse.tile as tile
from concourse import mybir
from concourse.bass_utils import run_bass_kernel_spmd

# ---- problem constants (hardcoded per harness contract) ----
B, D, N, V = 131072, 3, 48, 16
NCORES = 8
BS = B // NCORES  # 16384 queries per core
P = 128           # partitions
T = BS // P       # 128 tiles of 128 queries
NP = N + 2        # 50 (padded grid extent)
NI = N - 1        # 47 interval starts (idx in [0, 46])
NB = 4 * 4 * 4 * V       # 1024 f32 neighborhood per query
ROW = NB + 16            # + [3 dims x 4 knot vals] + 4 pad = 1040 f32 (4160B)
NROWS = NI * NI * NI     # 103823 gather-table rows
G = 16            # tiles per supertile (weight-batching granularity)
NST = T // G      # supertiles

F32 = mybir.dt.float32
I32 = mybir.dt.int32
OP = mybir.AluOpType

DEBUG = False
REPS = 1  # phase-B repetitions (differential HW timing; harness uses 1)


def _emit_kernel_general(nc: bass.Bass):
    xkn = nc.dram_tensor("xkn", [P, D * T + D * N], F32, kind="ExternalInput")
    gtab = nc.dram_tensor("gtab", [NROWS, ROW], F32, kind="ExternalInput")
    outb = nc.dram_tensor("outb", [P, T * V], F32, kind="ExternalOutput")
    if DEBUG:
        dbg_gidx = nc.dram_tensor("dbg_gidx", [P, T], I32, kind="ExternalOutput")
        dbg_nb = nc.dram_tensor("dbg_nb", [P, G * ROW], F32, kind="ExternalOutput")
        dbg_w = nc.dram_tensor("dbg_w", [P, D * 4 * G], F32, kind="ExternalOutput")

    with tile.TileContext(nc) as tc:
        from contextlib import ExitStack

        with ExitStack() as ctx:
            singles = ctx.enter_context(tc.tile_pool(name="singles", bufs=1))
            nbpool = ctx.enter_context(tc.tile_pool(name="nb", bufs=2 * G))
            wpool = ctx.enter_context(tc.tile_pool(name="wp", bufs=2))
            accpool = ctx.enter_context(tc.tile_pool(name="acc", bufs=4))

            # ---------------- load inputs (one DMA) ----------------
            sb_xk = singles.tile([P, D * T + D * N], F32, tag="sb_xk", name="sb_xk")
            nc.sync.dma_start(out=sb_xk[:], in_=xkn[:])
            sb_x = sb_xk[:, : D * T].rearrange("p (d t) -> p d t", d=D)
            sb_kn = sb_xk[:, D * T :].rearrange("p (d n) -> p d n", d=D)

            # ---------------- phase A: searchsorted + row index ----------------
            idx_f = [
                singles.tile([P, T], F32, tag=f"idx{d}", name=f"idx{d}")
                for d in range(D)
            ]
            for d in range(D):
                xd = sb_x[:, d, :]
                acc = idx_f[d]
                nc.vector.tensor_scalar(
                    out=acc[:], in0=xd, scalar1=sb_kn[:, d, 1:2], scalar2=None,
                    op0=OP.is_ge,
                )
                for j in range(2, N):
                    nc.vector.scalar_tensor_tensor(
                        out=acc[:], in0=xd, scalar=sb_kn[:, d, j : j + 1],
                        in1=acc[:], op0=OP.is_ge, op1=OP.add,
                    )
                nc.vector.tensor_scalar(
                    out=acc[:], in0=acc[:], scalar1=float(NI - 1), scalar2=None,
                    op0=OP.min,
                )

            # base row = idx0*NI*NI + idx1*NI + idx2
            basef = singles.tile([P, T], F32, tag="basef", name="basef")
            nc.vector.scalar_tensor_tensor(
                out=basef[:], in0=idx_f[1][:], scalar=float(NI), in1=idx_f[2][:],
                op0=OP.mult, op1=OP.add,
            )
            nc.vector.scalar_tensor_tensor(
                out=basef[:], in0=idx_f[0][:], scalar=float(NI * NI), in1=basef[:],
                op0=OP.mult, op1=OP.add,
            )
            gidx = singles.tile([P, T], I32, tag="gidx", name="gidx")
            nc.vector.tensor_copy(out=gidx[:], in_=basef[:])
            if DEBUG:
                nc.sync.dma_start(out=dbg_gidx[:], in_=gidx[:])

            # ---------------- phase B: gather + weights + contract ----------
            outbuf = singles.tile([P, T * V], F32, tag="outbuf", name="outbuf")
            for s in range(NST * REPS):
                s = s % NST
                nbs = [
                    nbpool.tile([P, ROW], F32, tag="nb", name=f"nb_{s}_{tl}")
                    for tl in range(G)
                ]
                for tl in range(G):
                    t = s * G + tl
                    nc.gpsimd.indirect_dma_start(
                        out=nbs[tl][:],
                        out_offset=None,
                        in_=gtab[:],
                        in_offset=bass.IndirectOffsetOnAxis(
                            ap=gidx[:, t : t + 1], axis=0
                        ),
                    )
                if DEBUG and s == 0:
                    for tl in range(G):
                        nc.sync.dma_start(
                            out=dbg_nb[:, tl * ROW : (tl + 1) * ROW], in_=nbs[tl][:]
                        )

                # collect the embedded knot windows: kvst[p, tl*16 + (d*4+c)]
                kvst = wpool.tile([P, G * 16], F32, tag="kvst", name="kvst")
                for tl in range(G):
                    nc.vector.tensor_copy(
                        out=kvst[:, tl * 16 : (tl + 1) * 16], in_=nbs[tl][:, NB:]
                    )

                # weights per dim, batched over the supertile: [P, G] ops
                w = [
                    [
                        wpool.tile([P, G], F32, tag=f"w{d}{c}", name=f"w{d}{c}")
                        for c in range(4)
                    ]
                    for d in range(D)
                ]
                scr = [
                    wpool.tile([P, G], F32, tag=f"scr{i}", name=f"scr{i}")
                    for i in range(6)
                ]
                u_t, u2_t, u3_t, s0, s1, s2 = scr
                for d in range(D):
                    xd = sb_x[:, d, s * G : (s + 1) * G]
                    kbase = kvst[:]

                    def kvv(c, _kb=kbase, _d=d):
                        return bass.AP(
                            kvst.tensor,
                            _kb.offset + _d * 4 + c,
                            [_kb.ap[0], [16, G]],
                        )

                    tm1, t0, t1, t2 = kvv(0), kvv(1), kvv(2), kvv(3)
                    dt = s0
                    nc.vector.tensor_sub(out=dt[:], in0=t1, in1=t0)
                    xm = s1
                    nc.vector.tensor_sub(out=xm[:], in0=xd, in1=t0)  # x - t0
                    rcp = s2
                    nc.vector.reciprocal(out=rcp[:], in_=dt[:])
                    nc.vector.tensor_mul(out=u_t[:], in0=xm[:], in1=rcp[:])  # u
                    nc.vector.tensor_mul(out=u2_t[:], in0=u_t[:], in1=u_t[:])
                    nc.vector.tensor_mul(out=u3_t[:], in0=u2_t[:], in1=u_t[:])
                    a_t = s1
                    nc.vector.tensor_sub(out=a_t[:], in0=t1, in1=tm1)
                    nc.vector.reciprocal(out=a_t[:], in_=a_t[:])
                    nc.vector.tensor_mul(out=a_t[:], in0=a_t[:], in1=dt[:])
                    b_t = s2
                    nc.vector.tensor_sub(out=b_t[:], in0=t2, in1=t0)
                    nc.vector.reciprocal(out=b_t[:], in_=b_t[:])
                    nc.vector.tensor_mul(out=b_t[:], in0=b_t[:], in1=dt[:])
                    # h00 = 2u3 - 3u2 + 1 ; h10 = u3 - 2u2 + u ; h11 = u3 - u2
                    h00 = w[d][1]
                    nc.vector.tensor_scalar(
                        out=h00[:], in0=u2_t[:], scalar1=-3.0, scalar2=1.0,
                        op0=OP.mult, op1=OP.add,
                    )
                    nc.vector.scalar_tensor_tensor(
                        out=h00[:], in0=u3_t[:], scalar=2.0, in1=h00[:],
                        op0=OP.mult, op1=OP.add,
                    )
                    h10 = w[d][0]
                    nc.vector.tensor_scalar(
                        out=h10[:], in0=u2_t[:], scalar1=-2.0, scalar2=None,
                        op0=OP.mult,
                    )
                    nc.vector.tensor_add(out=h10[:], in0=h10[:], in1=u_t[:])
                    nc.vector.tensor_add(out=h10[:], in0=h10[:], in1=u3_t[:])
                    h11 = w[d][3]
                    nc.vector.tensor_sub(out=h11[:], in0=u3_t[:], in1=u2_t[:])
                    p1 = w[d][0]
                    nc.vector.tensor_mul(out=p1[:], in0=h10[:], in1=a_t[:])
                    p2 = w[d][3]
                    nc.vector.tensor_mul(out=p2[:], in0=h11[:], in1=b_t[:])
                    # w2 = (p1 - h00) + 1 ; w1 = h00 - p2 ; w0 = -p1 ; w3 = p2
                    nc.vector.tensor_sub(out=w[d][2][:], in0=p1[:], in1=h00[:])
                    nc.vector.tensor_scalar(
                        out=w[d][2][:], in0=w[d][2][:], scalar1=1.0, scalar2=None,
                        op0=OP.add,
                    )
                    nc.vector.tensor_sub(out=w[d][1][:], in0=h00[:], in1=p2[:])
                    nc.vector.tensor_scalar(
                        out=w[d][0][:], in0=p1[:], scalar1=-1.0, scalar2=None,
                        op0=OP.mult,
                    )
                if DEBUG and s == 0:
                    for d in range(D):
                        for c in range(4):
                            nc.sync.dma_start(
                                out=dbg_w[
                                    :, (d * 4 + c) * G : (d * 4 + c + 1) * G
                                ],
                                in_=w[d][c][:],
                            )

                # contraction per tile: layout [i(4)x256, j(4)x64, k(4)x16, v(16)]
                for tl in range(G):
                    t = s * G + tl
                    tview = nbs[tl][:]
                    acc1 = accpool.tile([P, 256], F32, tag="acc1", name="acc1")
                    for k in range(4):
                        kslice = bass.AP(
                            nbs[tl].tensor,
                            tview.offset + k * V,
                            [tview.ap[0], [64, 16], [1, V]],
                        )
                        if k == 0:
                            nc.vector.tensor_scalar(
                                out=acc1[:], in0=kslice,
                                scalar1=w[2][k][:, tl : tl + 1],
                                scalar2=None, op0=OP.mult,
                            )
                        else:
                            nc.vector.scalar_tensor_tensor(
                                out=acc1[:], in0=kslice,
                                scalar=w[2][k][:, tl : tl + 1],
                                in1=acc1[:], op0=OP.mult, op1=OP.add,
                            )
                    acc2 = accpool.tile([P, 64], F32, tag="acc2", name="acc2")
                    for j in range(4):
                        jslice = bass.AP(
                            acc1.tensor,
                            acc1[:].offset + j * V,
                            [acc1[:].ap[0], [64, 4], [1, V]],
                        )
                        if j == 0:
                            nc.vector.tensor_scalar(
                                out=acc2[:], in0=jslice,
                                scalar1=w[1][j][:, tl : tl + 1],
                                scalar2=None, op0=OP.mult,
                            )
                        else:
                            nc.vector.scalar_tensor_tensor(
                                out=acc2[:], in0=jslice,
                                scalar=w[1][j][:, tl : tl + 1],
                                in1=acc2[:], op0=OP.mult, op1=OP.add,
                            )
                    oslice = outbuf[:, t * V : (t + 1) * V]
                    for i in range(4):
                        islice = acc2[:, i * V : (i + 1) * V]
                        if i == 0:
                            nc.vector.tensor_scalar(
                                out=oslice, in0=islice,
                                scalar1=w[0][i][:, tl : tl + 1],
                                scalar2=None, op0=OP.mult,
                            )
                        else:
                            nc.vector.scalar_tensor_tensor(
                                out=oslice, in0=islice,
                                scalar=w[0][i][:, tl : tl + 1],
                                in1=oslice, op0=OP.mult, op1=OP.add,
                            )
            nc.sync.dma_start(out=outb[:], in_=outbuf[:])
    return nc


def _emit_kernel_affine(nc: bass.Bass, k0s, invs):
    """Fast path for affine (uniform-spacing) knots: idx/u/weights are pure
    arithmetic (knot values baked as immediates), rows are exactly 1024 f32
    with layout [ij(16), v(16), k(4)] (k innermost)."""
    xkn = nc.dram_tensor("xkn", [P, D * T + D * N], F32, kind="ExternalInput")
    gtab = nc.dram_tensor("gtab", [NROWS, NB], F32, kind="ExternalInput")
    outb = nc.dram_tensor("outb", [P, T * V], F32, kind="ExternalOutput")

    with tile.TileContext(nc) as tc:
        from contextlib import ExitStack

        with ExitStack() as ctx:
            singles = ctx.enter_context(tc.tile_pool(name="singles", bufs=1))
            nbpool = ctx.enter_context(tc.tile_pool(name="nb", bufs=20))
            accpool = ctx.enter_context(tc.tile_pool(name="acc", bufs=2))
            p1pool = ctx.enter_context(tc.tile_pool(name="p1", bufs=2))

            sb_xk = singles.tile([P, D * T + D * N], F32, tag="sb_xk", name="sb_xk")
            nc.sync.dma_start(out=sb_xk[:], in_=xkn[:])
            sb_x = sb_xk[:, : D * T].rearrange("p (d t) -> p d t", d=D)

            # ---- phase A: idx, u, weights (global, arithmetic) ----
            idx_f = [
                singles.tile([P, T], F32, tag=f"idx{d}", name=f"idx{d}")
                for d in range(D)
            ]
            u_d = [
                singles.tile([P, T], F32, tag=f"u{d}", name=f"u{d}") for d in range(D)
            ]
            scr = [
                singles.tile([P, T], F32, tag=f"sc{i}", name=f"sc{i}") for i in range(4)
            ]
            tA, f_t, u2_t, u3_t = scr
            # wd[d] layout [P, (t,c)]: column t*4+c
            wd = [
                singles.tile([P, T * 4], F32, tag=f"wd{d}", name=f"wd{d}")
                for d in range(D)
            ]
            for d in range(D):
                xd = sb_x[:, d, :]
                nc.vector.tensor_scalar(
                    out=tA[:], in0=xd, scalar1=-float(k0s[d]), scalar2=float(invs[d]),
                    op0=OP.add, op1=OP.mult,
                )
                # floor(t) for t in (-1, 48): round(t - 0.5) via the 2^23
                # magic-number trick (one dual-op instruction)
                nc.vector.tensor_scalar(
                    out=idx_f[d][:], in0=tA[:], scalar1=8388607.5,
                    scalar2=8388608.0, op0=OP.add, op1=OP.subtract,
                )
                nc.vector.tensor_scalar(
                    out=idx_f[d][:], in0=idx_f[d][:], scalar1=float(NI - 1),
                    scalar2=0.0, op0=OP.min, op1=OP.max,
                )
                nc.vector.tensor_sub(out=u_d[d][:], in0=tA[:], in1=idx_f[d][:])

            basef = singles.tile([P, T], F32, tag="basef", name="basef")
            nc.vector.scalar_tensor_tensor(
                out=basef[:], in0=idx_f[1][:], scalar=float(NI), in1=idx_f[2][:],
                op0=OP.mult, op1=OP.add,
            )
            nc.vector.scalar_tensor_tensor(
                out=basef[:], in0=idx_f[0][:], scalar=float(NI * NI), in1=basef[:],
                op0=OP.mult, op1=OP.add,
            )
            gidx = singles.tile([P, T], I32, tag="gidx", name="gidx")
            nc.vector.tensor_copy(out=gidx[:], in_=basef[:])

            for d in range(D):
                u = u_d[d]
                nc.vector.tensor_mul(out=u2_t[:], in0=u[:], in1=u[:])
                nc.vector.tensor_mul(out=u3_t[:], in0=u2_t[:], in1=u[:])
                h00 = idx_f[d]  # reuse (idx no longer needed for this dim)
                nc.vector.tensor_scalar(
                    out=h00[:], in0=u2_t[:], scalar1=-3.0, scalar2=1.0,
                    op0=OP.mult, op1=OP.add,
                )
                nc.vector.scalar_tensor_tensor(
                    out=h00[:], in0=u3_t[:], scalar=2.0, in1=h00[:],
                    op0=OP.mult, op1=OP.add,
                )
                h10 = tA
                nc.vector.tensor_scalar(
                    out=h10[:], in0=u2_t[:], scalar1=-2.0, scalar2=None, op0=OP.mult
                )
                nc.vector.tensor_add(out=h10[:], in0=h10[:], in1=u[:])
                nc.vector.tensor_add(out=h10[:], in0=h10[:], in1=u3_t[:])
                h11 = f_t
                nc.vector.tensor_sub(out=h11[:], in0=u3_t[:], in1=u2_t[:])

                def wcol(c, _d=d):
                    base_ap = wd[_d][:]
                    return bass.AP(wd[_d].tensor, base_ap.offset + c, [base_ap.ap[0], [4, T]])

                # w0 = -0.5*h10 ; w3 = 0.5*h11 ; w1 = h00 - 0.5*h11 ; w2 = 1 - h00 + 0.5*h10
                nc.vector.tensor_scalar(
                    out=wcol(0), in0=h10[:], scalar1=-0.5, scalar2=None, op0=OP.mult
                )
                nc.vector.tensor_scalar(
                    out=wcol(3), in0=h11[:], scalar1=0.5, scalar2=None, op0=OP.mult
                )
                nc.vector.scalar_tensor_tensor(
                    out=wcol(1), in0=h11[:], scalar=-0.5, in1=h00[:],
                    op0=OP.mult, op1=OP.add,
                )
                tmp3 = u2_t
                nc.vector.tensor_scalar(
                    out=tmp3[:], in0=h00[:], scalar1=-1.0, scalar2=1.0,
                    op0=OP.mult, op1=OP.add,
                )
                nc.vector.scalar_tensor_tensor(
                    out=wcol(2), in0=h10[:], scalar=0.5, in1=tmp3[:],
                    op0=OP.mult, op1=OP.add,
                )

            # W12[p, t*16 + i*4 + j] = wd0[t,i] * wd1[t,j]
            w12 = singles.tile([P, T * 16], F32, tag="w12", name="w12")
            for i in range(4):
                nc.vector.tensor_tensor(
                    out=bass.AP(w12.tensor, w12[:].offset + i * 4,
                                [w12[:].ap[0], [16, T], [1, 4]]),
                    in0=bass.AP(wd[1].tensor, wd[1][:].offset,
                                [wd[1][:].ap[0], [4, T], [1, 4]]),
                    in1=bass.AP(wd[0].tensor, wd[0][:].offset + i,
                                [wd[0][:].ap[0], [4, T], [0, 4]]),
                    op=OP.mult,
                )

            # ---- phase B ----
            outbuf = singles.tile([P, T * V], F32, tag="outbuf", name="outbuf")
            for s in range(NST * REPS):
                s = s % NST
                nbs = [
                    nbpool.tile([P, NB], F32, tag="nb", name=f"nb_{s}_{tl}")
                    for tl in range(G)
                ]
                for tl in range(G):
                    t = s * G + tl
                    nc.gpsimd.indirect_dma_start(
                        out=nbs[tl][:],
                        out_offset=None,
                        in_=gtab[:],
                        in_offset=bass.IndirectOffsetOnAxis(
                            ap=gidx[:, t : t + 1], axis=0
                        ),
                    )
                acc = accpool.tile([P, G * 256], F32, tag="acct", name="acct")
                for tl in range(G):
                    t = s * G + tl
                    tv = nbs[tl][:]
                    for k in range(4):
                        kslice = bass.AP(
                            nbs[tl].tensor, tv.offset + k * 256, [tv.ap[0], [1, 256]]
                        )
                        oslice = acc[:, tl * 256 : (tl + 1) * 256]
                        sc = bass.AP(
                            wd[2].tensor, wd[2][:].offset + t * 4 + k,
                            [wd[2][:].ap[0], [1, 1]],
                        )
                        if k == 0:
                            nc.vector.tensor_scalar(
                                out=oslice, in0=kslice, scalar1=sc, scalar2=None,
                                op0=OP.mult,
                            )
                        else:
                            nc.vector.scalar_tensor_tensor(
                                out=oslice, in0=kslice, scalar=sc, in1=oslice,
                                op0=OP.mult, op1=OP.add,
                            )
                # stage 2 (batched over supertile):
                # P1v[t, v, ij] = acc[t, ij, v] * W12[t, ij]
                p1v = p1pool.tile([P, G * 256], F32, tag="p1v", name="p1v")
                nc.vector.tensor_tensor(
                    out=bass.AP(p1v.tensor, p1v[:].offset,
                                [p1v[:].ap[0], [256, G], [16, 16], [1, 16]]),
                    in0=bass.AP(acc.tensor, acc[:].offset,
                                [acc[:].ap[0], [256, G], [1, 16], [16, 16]]),
                    in1=bass.AP(w12.tensor, w12[:].offset + s * G * 16,
                                [w12[:].ap[0], [16, G], [0, 16], [1, 16]]),
                    op=OP.mult,
                )
                # note: iteration dims above are (t, v, ij) for out;
                # in0 iterates (t, v, ij) -> acc[t, ij, v] via strides (256,1,16)
                nc.vector.tensor_reduce(
                    out=outbuf[:, s * G * V : (s + 1) * G * V],
                    in_=bass.AP(p1v.tensor, p1v[:].offset,
                                [p1v[:].ap[0], [16, G * 16], [1, 16]]),
                    axis=mybir.AxisListType.X,
                    op=OP.add,
                )
            nc.sync.dma_start(out=outb[:], in_=outbuf[:])
    return nc


_NC_CACHE = {}


def _affine_params(knots):
    """Return (k0s, invs) if each dim's knots are (near-)affine, else None."""
    kn = np.asarray(knots, dtype=np.float64)
    k0s, invs = [], []
    for d in range(D):
        kd = kn[d]
        step = (kd[-1] - kd[0]) / (N - 1)
        if step <= 0:
            return None
        fit = kd[0] + step * np.arange(N)
        if not np.allclose(kd, fit, rtol=0, atol=1e-6 * max(1.0, abs(step) * N)):
            return None
        k0s.append(float(kd[0]))
        invs.append(float(1.0 / step))
    return k0s, invs


def _get_nc(mode="general", params=None):
    key = (mode, tuple(params[0]) + tuple(params[1]) if params else None, REPS)
    if key not in _NC_CACHE:
        nc = bacc.Bacc(None, target_bir_lowering=False)
        if mode == "affine":
            _emit_kernel_affine(nc, params[0], params[1])
        else:
            _emit_kernel_general(nc)
        nc.compile()
        _NC_CACHE[key] = nc
    return _NC_CACHE[key]


_PREP_CACHE = {}


def _host_prep(knots, grid, affine=False):
    key = (id(knots), id(grid), affine)
    if key in _PREP_CACHE:
        return _PREP_CACHE[key]
    from numpy.lib.stride_tricks import sliding_window_view

    gp = np.asarray(grid, dtype=np.float32)
    for ax in range(3):
        lo = 2.0 * np.take(gp, 0, axis=ax) - np.take(gp, 1, axis=ax)
        hi = 2.0 * np.take(gp, -1, axis=ax) - np.take(gp, -2, axis=ax)
        gp = np.concatenate(
            [np.expand_dims(lo, ax), gp, np.expand_dims(hi, ax)], axis=ax
        )
    sw = sliding_window_view(gp, (4, 4, 4), axis=(0, 1, 2))
    if affine:
        # rows [47,47,47, k4, (ij)16, v16] = 1024 f32 (4KB): per-k contiguous
        nbr = np.ascontiguousarray(
            sw.transpose(0, 1, 2, 6, 4, 5, 3)  # [.., k, i, j, v]
        ).reshape(NROWS, NB)
        kn = np.asarray(knots, dtype=np.float32)
        knr = np.ascontiguousarray(
            np.broadcast_to(kn.reshape(1, D * N), (P, D * N))
        ).astype(np.float32)
        _PREP_CACHE[key] = (nbr, knr)
        return nbr, knr
    # general path: rows [.., i, j, k, v] + embedded knot windows
    nbr = sw.transpose(0, 1, 2, 4, 5, 6, 3).reshape(NROWS, NB)
    # knot windows per dim: ktw[d][i] = padded_knots[d][i:i+4]
    kn = np.asarray(knots, dtype=np.float32)
    tp = np.concatenate(
        [2.0 * kn[:, :1] - kn[:, 1:2], kn, 2.0 * kn[:, -1:] - kn[:, -2:-1]], axis=1
    )
    ktw = sliding_window_view(tp, 4, axis=1)  # [3, 47, 4]
    gtab = np.empty((NROWS, ROW), dtype=np.float32)
    gtab[:, :NB] = nbr
    kv = gtab[:, NB : NB + 12].reshape(NI, NI, NI, 3, 4)
    kv[:, :, :, 0, :] = ktw[0][:, None, None, :]
    kv[:, :, :, 1, :] = ktw[1][None, :, None, :]
    kv[:, :, :, 2, :] = ktw[2][None, None, :, :]
    gtab[:, NB + 12 :] = 0.0
    knr = np.ascontiguousarray(
        np.broadcast_to(kn.reshape(1, D * N), (P, D * N))
    ).astype(np.float32)
    _PREP_CACHE[key] = (gtab, knr)
    return gtab, knr


def kernel(x, knots, grid):
    x = np.asarray(x, dtype=np.float32)
    params = _affine_params(knots)
    if params is not None:
        gtab, knr = _host_prep(knots, grid, affine=True)
        nc = _get_nc("affine", params)
    else:
        gtab, knr = _host_prep(knots, grid)
        nc = _get_nc()
    in_maps = []
    for c in range(NCORES):
        xs = x[c * BS : (c + 1) * BS]  # [BS, 3]
        xTc = np.ascontiguousarray(
            xs.reshape(T, P, D).transpose(1, 2, 0).reshape(P, D * T)
        )  # [p, d*T + t] = x[t*P + p, d]
        xknc = np.concatenate([xTc, knr], axis=1)
        in_maps.append({"xkn": xknc, "gtab": gtab})
    res = run_bass_kernel_spmd(nc, in_maps, core_ids=list(range(NCORES)))
    outs = []
    for c in range(NCORES):
        ob = res.results[c]["outb"].reshape(P, T, V)  # [p, t, v]
        outs.append(np.ascontiguousarray(ob.transpose(1, 0, 2)).reshape(BS, V))
    return np.concatenate(outs, axis=0)


# revision 17
# speedup vs baseline: 1.6127x; 1.2258x over previous
"""Trainium2 Bass kernel for non-uniform 3D Catmull-Rom spline interpolation.

Problem: x [131072, 3] query points, knots [3, 48], grid [48,48,48,16]
-> out [131072, 16].

Strategy (data-parallel over the batch across 8 cores):
  Host: pad the grid to [50,50,50,16]; build a replicated gather table
    gtab[(p0, q0, r0), 1040] whose 4160B rows hold the full 4x4x4x16
    neighborhood gp[p0:p0+4, q0:q0+4, r0:r0+4, :] (1024 f32) plus the 12
    knot-window values [tm1,t0,t1,t2] per dim (keyed by p0/q0/r0), padded
    to 1040 f32.  A query's entire working set is then ONE contiguous 4KB+
    row -> one [128,1]-index indirect DMA per 128-query tile (the only
    reliably-ordered indirect-DMA shape on TRN2 SWDGE; >=512B descriptors
    run at full DMA rate).
  Device (per core, 16384 queries = 128 tiles of 128 queries on partitions):
    - searchsorted per dim via 47 fused (x >= knot_j) + acc ops (batched
      over all queries)
    - row index = idx0*47*47 + idx1*47 + idx2 -> gidx [128,128] int32
    - per supertile of G tiles: G gathers, Catmull-Rom weights from the
      embedded knot windows (batched [128,G] DVE ops), then per-tile
      contraction k -> j -> i with tensor_scalar / scalar_tensor_tensor
      fused multiply-adds using per-partition scalars (per-query weights)
"""

import numpy as np

import concourse.bass as bass
import concourse.bacc as bacc
import concourse.tile as tile
from concourse import mybir
from concourse.bass_utils import run_bass_kernel_spmd

# ---- problem constants (hardcoded per harness contract) ----
B, D, N, V = 131072, 3, 48, 16
NCORES = 8
BS = B // NCORES  # 16384 queries per core
P = 128           # partitions
T = BS // P       # 128 tiles of 128 queries
NP = N + 2        # 50 (padded grid extent)
NI = N - 1        # 47 interval starts (idx in [0, 46])
NB = 4 * 4 * 4 * V       # 1024 f32 neighborhood per query
ROW = NB + 16            # + [3 dims x 4 knot vals] + 4 pad = 1040 f32 (4160B)
NROWS = NI * NI * NI     # 103823 gather-table rows
G = 16            # tiles per supertile (weight-batching granularity)
NST = T // G      # supertiles

F32 = mybir.dt.float32
I32 = mybir.dt.int32
OP = mybir.AluOpType

DEBUG = False
REPS = 1  # phase-B repetitions (differential HW timing; harness uses 1)


def _emit_kernel_general(nc: bass.Bass):
    xkn = nc.dram_tensor("xkn", [P, D * T + D * N], F32, kind="ExternalInput")
    gtab = nc.dram_tensor("gtab", [NROWS, ROW], F32, kind="ExternalInput")
    outb = nc.dram_tensor("outb", [P, T * V], F32, kind="ExternalOutput")
    if DEBUG:
        dbg_gidx = nc.dram_tensor("dbg_gidx", [P, T], I32, kind="ExternalOutput")
        dbg_nb = nc.dram_tensor("dbg_nb", [P, G * ROW], F32, kind="ExternalOutput")
        dbg_w = nc.dram_tensor("dbg_w", [P, D * 4 * G], F32, kind="ExternalOutput")

    with tile.TileContext(nc) as tc:
        from contextlib import ExitStack

        with ExitStack() as ctx:
            singles = ctx.enter_context(tc.tile_pool(name="singles", bufs=1))
            nbpool = ctx.enter_context(tc.tile_pool(name="nb", bufs=2 * G))
            wpool = ctx.enter_context(tc.tile_pool(name="wp", bufs=2))
            accpool = ctx.enter_context(tc.tile_pool(name="acc", bufs=4))

            # ---------------- load inputs (one DMA) ----------------
            sb_xk = singles.tile([P, D * T + D * N], F32, tag="sb_xk", name="sb_xk")
            nc.sync.dma_start(out=sb_xk[:], in_=xkn[:])
            sb_x = sb_xk[:, : D * T].rearrange("p (d t) -> p d t", d=D)
            sb_kn = sb_xk[:, D * T :].rearrange("p (d n) -> p d n", d=D)

            # ---------------- phase A: searchsorted + row index ----------------
            idx_f = [
                singles.tile([P, T], F32, tag=f"idx{d}", name=f"idx{d}")
                for d in range(D)
            ]
            for d in range(D):
                xd = sb_x[:, d, :]
                acc = idx_f[d]
                nc.vector.tensor_scalar(
                    out=acc[:], in0=xd, scalar1=sb_kn[:, d, 1:2], scalar2=None,
                    op0=OP.is_ge,
                )
                for j in range(2, N):
                    nc.vector.scalar_tensor_tensor(
                        out=acc[:], in0=xd, scalar=sb_kn[:, d, j : j + 1],
                        in1=acc[:], op0=OP.is_ge, op1=OP.add,
                    )
                nc.vector.tensor_scalar(
                    out=acc[:], in0=acc[:], scalar1=float(NI - 1), scalar2=None,
                    op0=OP.min,
                )

            # base row = idx0*NI*NI + idx1*NI + idx2
            basef = singles.tile([P, T], F32, tag="basef", name="basef")
            nc.vector.scalar_tensor_tensor(
                out=basef[:], in0=idx_f[1][:], scalar=float(NI), in1=idx_f[2][:],
                op0=OP.mult, op1=OP.add,
            )
            nc.vector.scalar_tensor_tensor(
                out=basef[:], in0=idx_f[0][:], scalar=float(NI * NI), in1=basef[:],
                op0=OP.mult, op1=OP.add,
            )
            gidx = singles.tile([P, T], I32, tag="gidx", name="gidx")
            nc.vector.tensor_copy(out=gidx[:], in_=basef[:])
            if DEBUG:
                nc.sync.dma_start(out=dbg_gidx[:], in_=gidx[:])

            # ---------------- phase B: gather + weights + contract ----------
            outbuf = singles.tile([P, T * V], F32, tag="outbuf", name="outbuf")
            for s in range(NST * REPS):
                s = s % NST
                nbs = [
                    nbpool.tile([P, ROW], F32, tag="nb", name=f"nb_{s}_{tl}")
                    for tl in range(G)
                ]
                for tl in range(G):
                    t = s * G + tl
                    nc.gpsimd.indirect_dma_start(
                        out=nbs[tl][:],
                        out_offset=None,
                        in_=gtab[:],
                        in_offset=bass.IndirectOffsetOnAxis(
                            ap=gidx[:, t : t + 1], axis=0
                        ),
                    )
                if DEBUG and s == 0:
                    for tl in range(G):
                        nc.sync.dma_start(
                            out=dbg_nb[:, tl * ROW : (tl + 1) * ROW], in_=nbs[tl][:]
                        )

                # collect the embedded knot windows: kvst[p, tl*16 + (d*4+c)]
                kvst = wpool.tile([P, G * 16], F32, tag="kvst", name="kvst")
                for tl in range(G):
                    nc.vector.tensor_copy(
                        out=kvst[:, tl * 16 : (tl + 1) * 16], in_=nbs[tl][:, NB:]
                    )

                # weights per dim, batched over the supertile: [P, G] ops
                w = [
                    [
                        wpool.tile([P, G], F32, tag=f"w{d}{c}", name=f"w{d}{c}")
                        for c in range(4)
                    ]
                    for d in range(D)
                ]
                scr = [
                    wpool.tile([P, G], F32, tag=f"scr{i}", name=f"scr{i}")
                    for i in range(6)
                ]
                u_t, u2_t, u3_t, s0, s1, s2 = scr
                for d in range(D):
                    xd = sb_x[:, d, s * G : (s + 1) * G]
                    kbase = kvst[:]

                    def kvv(c, _kb=kbase, _d=d):
                        return bass.AP(
                            kvst.tensor,
                            _kb.offset + _d * 4 + c,
                            [_kb.ap[0], [16, G]],
                        )

                    tm1, t0, t1, t2 = kvv(0), kvv(1), kvv(2), kvv(3)
                    dt = s0
                    nc.vector.tensor_sub(out=dt[:], in0=t1, in1=t0)
                    xm = s1
                    nc.vector.tensor_sub(out=xm[:], in0=xd, in1=t0)  # x - t0
                    rcp = s2
                    nc.vector.reciprocal(out=rcp[:], in_=dt[:])
                    nc.vector.tensor_mul(out=u_t[:], in0=xm[:], in1=rcp[:])  # u
                    nc.vector.tensor_mul(out=u2_t[:], in0=u_t[:], in1=u_t[:])
                    nc.vector.tensor_mul(out=u3_t[:], in0=u2_t[:], in1=u_t[:])
                    a_t = s1
                    nc.vector.tensor_sub(out=a_t[:], in0=t1, in1=tm1)
                    nc.vector.reciprocal(out=a_t[:], in_=a_t[:])
                    nc.vector.tensor_mul(out=a_t[:], in0=a_t[:], in1=dt[:])
                    b_t = s2
                    nc.vector.tensor_sub(out=b_t[:], in0=t2, in1=t0)
                    nc.vector.reciprocal(out=b_t[:], in_=b_t[:])
                    nc.vector.tensor_mul(out=b_t[:], in0=b_t[:], in1=dt[:])
                    # h00 = 2u3 - 3u2 + 1 ; h10 = u3 - 2u2 + u ; h11 = u3 - u2
                    h00 = w[d][1]
                    nc.vector.tensor_scalar(
                        out=h00[:], in0=u2_t[:], scalar1=-3.0, scalar2=1.0,
                        op0=OP.mult, op1=OP.add,
                    )
                    nc.vector.scalar_tensor_tensor(
                        out=h00[:], in0=u3_t[:], scalar=2.0, in1=h00[:],
                        op0=OP.mult, op1=OP.add,
                    )
                    h10 = w[d][0]
                    nc.vector.tensor_scalar(
                        out=h10[:], in0=u2_t[:], scalar1=-2.0, scalar2=None,
                        op0=OP.mult,
                    )
                    nc.vector.tensor_add(out=h10[:], in0=h10[:], in1=u_t[:])
                    nc.vector.tensor_add(out=h10[:], in0=h10[:], in1=u3_t[:])
                    h11 = w[d][3]
                    nc.vector.tensor_sub(out=h11[:], in0=u3_t[:], in1=u2_t[:])
                    p1 = w[d][0]
                    nc.vector.tensor_mul(out=p1[:], in0=h10[:], in1=a_t[:])
                    p2 = w[d][3]
                    nc.vector.tensor_mul(out=p2[:], in0=h11[:], in1=b_t[:])
                    # w2 = (p1 - h00) + 1 ; w1 = h00 - p2 ; w0 = -p1 ; w3 = p2
                    nc.vector.tensor_sub(out=w[d][2][:], in0=p1[:], in1=h00[:])
                    nc.vector.tensor_scalar(
                        out=w[d][2][:], in0=w[d][2][:], scalar1=1.0, scalar2=None,
                        op0=OP.add,
                    )
                    nc.vector.tensor_sub(out=w[d][1][:], in0=h00[:], in1=p2[:])
                    nc.vector.tensor_scalar(
                        out=w[d][0][:], in0=p1[:], scalar1=-1.0, scalar2=None,
                        op0=OP.mult,
                    )
                if DEBUG and s == 0:
                    for d in range(D):
                        for c in range(4):
                            nc.sync.dma_start(
                                out=dbg_w[
                                    :, (d * 4 + c) * G : (d * 4 + c + 1) * G
                                ],
                                in_=w[d][c][:],
                            )

                # contraction per tile: layout [i(4)x256, j(4)x64, k(4)x16, v(16)]
                for tl in range(G):
                    t = s * G + tl
                    tview = nbs[tl][:]
                    acc1 = accpool.tile([P, 256], F32, tag="acc1", name="acc1")
                    for k in range(4):
                        kslice = bass.AP(
                            nbs[tl].tensor,
                            tview.offset + k * V,
                            [tview.ap[0], [64, 16], [1, V]],
                        )
                        if k == 0:
                            nc.vector.tensor_scalar(
                                out=acc1[:], in0=kslice,
                                scalar1=w[2][k][:, tl : tl + 1],
                                scalar2=None, op0=OP.mult,
                            )
                        else:
                            nc.vector.scalar_tensor_tensor(
                                out=acc1[:], in0=kslice,
                                scalar=w[2][k][:, tl : tl + 1],
                                in1=acc1[:], op0=OP.mult, op1=OP.add,
                            )
                    acc2 = accpool.tile([P, 64], F32, tag="acc2", name="acc2")
                    for j in range(4):
                        jslice = bass.AP(
                            acc1.tensor,
                            acc1[:].offset + j * V,
                            [acc1[:].ap[0], [64, 4], [1, V]],
                        )
                        if j == 0:
                            nc.vector.tensor_scalar(
                                out=acc2[:], in0=jslice,
                                scalar1=w[1][j][:, tl : tl + 1],
                                scalar2=None, op0=OP.mult,
                            )
                        else:
                            nc.vector.scalar_tensor_tensor(
                                out=acc2[:], in0=jslice,
                                scalar=w[1][j][:, tl : tl + 1],
                                in1=acc2[:], op0=OP.mult, op1=OP.add,
                            )
                    oslice = outbuf[:, t * V : (t + 1) * V]
                    for i in range(4):
                        islice = acc2[:, i * V : (i + 1) * V]
                        if i == 0:
                            nc.vector.tensor_scalar(
                                out=oslice, in0=islice,
                                scalar1=w[0][i][:, tl : tl + 1],
                                scalar2=None, op0=OP.mult,
                            )
                        else:
                            nc.vector.scalar_tensor_tensor(
                                out=oslice, in0=islice,
                                scalar=w[0][i][:, tl : tl + 1],
                                in1=oslice, op0=OP.mult, op1=OP.add,
                            )
            nc.sync.dma_start(out=outb[:], in_=outbuf[:])
    return nc


def _emit_kernel_affine(nc: bass.Bass, k0s, invs):
    """Fast path for affine (uniform-spacing) knots: idx/u/weights are pure
    arithmetic (knot values baked as immediates), rows are exactly 1024 f32
    with layout [ij(16), v(16), k(4)] (k innermost)."""
    xkn = nc.dram_tensor("xkn", [P, D * T + D * N], F32, kind="ExternalInput")
    gtab = nc.dram_tensor("gtab", [NROWS, NB], F32, kind="ExternalInput")
    outb = nc.dram_tensor("outb", [P, T * V], F32, kind="ExternalOutput")

    with tile.TileContext(nc) as tc:
        from contextlib import ExitStack

        with ExitStack() as ctx:
            singles = ctx.enter_context(tc.tile_pool(name="singles", bufs=1))
            nbpool = ctx.enter_context(tc.tile_pool(name="nb", bufs=20))
            accpool = ctx.enter_context(tc.tile_pool(name="acc", bufs=2))
            p1pool = ctx.enter_context(tc.tile_pool(name="p1", bufs=2))

            sb_xk = singles.tile([P, D * T + D * N], F32, tag="sb_xk", name="sb_xk")
            nc.sync.dma_start(out=sb_xk[:], in_=xkn[:])
            sb_x = sb_xk[:, : D * T].rearrange("p (d t) -> p d t", d=D)

            # ---- phase A: idx, u, weights (global, arithmetic) ----
            idx_f = [
                singles.tile([P, T], F32, tag=f"idx{d}", name=f"idx{d}")
                for d in range(D)
            ]
            u_d = [
                singles.tile([P, T], F32, tag=f"u{d}", name=f"u{d}") for d in range(D)
            ]
            scr = [
                singles.tile([P, T], F32, tag=f"sc{i}", name=f"sc{i}") for i in range(4)
            ]
            tA, f_t, u2_t, u3_t = scr
            # wd[d] layout [P, (t,c)]: column t*4+c
            wd = [
                singles.tile([P, T * 4], F32, tag=f"wd{d}", name=f"wd{d}")
                for d in range(D)
            ]
            for d in range(D):
                xd = sb_x[:, d, :]
                nc.vector.tensor_scalar(
                    out=tA[:], in0=xd, scalar1=-float(k0s[d]), scalar2=float(invs[d]),
                    op0=OP.add, op1=OP.mult,
                )
                # floor(t) for t in (-1, 48): round(t - 0.5) via the 2^23
                # magic-number trick (one dual-op instruction)
                nc.vector.tensor_scalar(
                    out=idx_f[d][:], in0=tA[:], scalar1=8388607.5,
                    scalar2=8388608.0, op0=OP.add, op1=OP.subtract,
                )
                nc.vector.tensor_scalar(
                    out=idx_f[d][:], in0=idx_f[d][:], scalar1=float(NI - 1),
                    scalar2=0.0, op0=OP.min, op1=OP.max,
                )
                nc.vector.tensor_sub(out=u_d[d][:], in0=tA[:], in1=idx_f[d][:])

            basef = singles.tile([P, T], F32, tag="basef", name="basef")
            nc.vector.scalar_tensor_tensor(
                out=basef[:], in0=idx_f[1][:], scalar=float(NI), in1=idx_f[2][:],
                op0=OP.mult, op1=OP.add,
            )
            nc.vector.scalar_tensor_tensor(
                out=basef[:], in0=idx_f[0][:], scalar=float(NI * NI), in1=basef[:],
                op0=OP.mult, op1=OP.add,
            )
            gidx = singles.tile([P, T], I32, tag="gidx", name="gidx")
            nc.vector.tensor_copy(out=gidx[:], in_=basef[:])

            for d in range(D):
                u = u_d[d]
                nc.vector.tensor_mul(out=u2_t[:], in0=u[:], in1=u[:])
                nc.vector.tensor_mul(out=u3_t[:], in0=u2_t[:], in1=u[:])
                h00 = idx_f[d]  # reuse (idx no longer needed for this dim)
                nc.vector.tensor_scalar(
                    out=h00[:], in0=u2_t[:], scalar1=-3.0, scalar2=1.0,
                    op0=OP.mult, op1=OP.add,
                )
                nc.vector.scalar_tensor_tensor(
                    out=h00[:], in0=u3_t[:], scalar=2.0, in1=h00[:],
                    op0=OP.mult, op1=OP.add,
                )
                h10 = tA
                nc.vector.tensor_scalar(
                    out=h10[:], in0=u2_t[:], scalar1=-2.0, scalar2=None, op0=OP.mult
                )
                nc.vector.tensor_add(out=h10[:], in0=h10[:], in1=u[:])
                nc.vector.tensor_add(out=h10[:], in0=h10[:], in1=u3_t[:])
                h11 = f_t
                nc.vector.tensor_sub(out=h11[:], in0=u3_t[:], in1=u2_t[:])

                def wcol(c, _d=d):
                    base_ap = wd[_d][:]
                    return bass.AP(wd[_d].tensor, base_ap.offset + c, [base_ap.ap[0], [4, T]])

                # w0 = -0.5*h10 ; w3 = 0.5*h11 ; w1 = h00 - 0.5*h11 ; w2 = 1 - h00 + 0.5*h10
                nc.vector.tensor_scalar(
                    out=wcol(0), in0=h10[:], scalar1=-0.5, scalar2=None, op0=OP.mult
                )
                nc.vector.tensor_scalar(
                    out=wcol(3), in0=h11[:], scalar1=0.5, scalar2=None, op0=OP.mult
                )
                nc.vector.scalar_tensor_tensor(
                    out=wcol(1), in0=h11[:], scalar=-0.5, in1=h00[:],
                    op0=OP.mult, op1=OP.add,
                )
                tmp3 = u2_t
                nc.vector.tensor_scalar(
                    out=tmp3[:], in0=h00[:], scalar1=-1.0, scalar2=1.0,
                    op0=OP.mult, op1=OP.add,
                )
                nc.vector.scalar_tensor_tensor(
                    out=wcol(2), in0=h10[:], scalar=0.5, in1=tmp3[:],
                    op0=OP.mult, op1=OP.add,
                )

            # W12[p, t*16 + i*4 + j] = wd0[t,i] * wd1[t,j]
            w12 = singles.tile([P, T * 16], F32, tag="w12", name="w12")
            for i in range(4):
                nc.vector.tensor_tensor(
                    out=bass.AP(w12.tensor, w12[:].offset + i * 4,
                                [w12[:].ap[0], [16, T], [1, 4]]),
                    in0=bass.AP(wd[1].tensor, wd[1][:].offset,
                                [wd[1][:].ap[0], [4, T], [1, 4]]),
                    in1=bass.AP(wd[0].tensor, wd[0][:].offset + i,
                                [wd[0][:].ap[0], [4, T], [0, 4]]),
                    op=OP.mult,
                )

            # ---- phase B ----
            outbuf = singles.tile([P, T * V], F32, tag="outbuf", name="outbuf")
            for s in range(NST * REPS):
                s = s % NST
                nbs = [
                    nbpool.tile([P, NB], F32, tag="nb", name=f"nb_{s}_{tl}")
                    for tl in range(G)
                ]
                for tl in range(G):
                    t = s * G + tl
                    nc.gpsimd.indirect_dma_start(
                        out=nbs[tl][:],
                        out_offset=None,
                        in_=gtab[:],
                        in_offset=bass.IndirectOffsetOnAxis(
                            ap=gidx[:, t : t + 1], axis=0
                        ),
                    )
                acc = accpool.tile([P, G * 256], F32, tag="acct", name="acct")
                for tl in range(G):
                    t = s * G + tl
                    tv = nbs[tl][:]
                    for k in range(4):
                        kslice = bass.AP(
                            nbs[tl].tensor, tv.offset + k * 256, [tv.ap[0], [1, 256]]
                        )
                        oslice = acc[:, tl * 256 : (tl + 1) * 256]
                        sc = bass.AP(
                            wd[2].tensor, wd[2][:].offset + t * 4 + k,
                            [wd[2][:].ap[0], [1, 1]],
                        )
                        if k == 0:
                            nc.vector.tensor_scalar(
                                out=oslice, in0=kslice, scalar1=sc, scalar2=None,
                                op0=OP.mult,
                            )
                        else:
                            nc.vector.scalar_tensor_tensor(
                                out=oslice, in0=kslice, scalar=sc, in1=oslice,
                                op0=OP.mult, op1=OP.add,
                            )
                # stage 2 (batched over supertile):
                # acc layout is (t, v, ij); P1v[t, v, ij] = acc * W12[t, ij]
                p1v = p1pool.tile([P, G * 256], F32, tag="p1v", name="p1v")
                nc.vector.tensor_tensor(
                    out=bass.AP(p1v.tensor, p1v[:].offset,
                                [p1v[:].ap[0], [256, G], [16, 16], [1, 16]]),
                    in0=bass.AP(acc.tensor, acc[:].offset,
                                [acc[:].ap[0], [256, G], [16, 16], [1, 16]]),
                    in1=bass.AP(w12.tensor, w12[:].offset + s * G * 16,
                                [w12[:].ap[0], [16, G], [0, 16], [1, 16]]),
                    op=OP.mult,
                )
                nc.vector.tensor_reduce(
                    out=outbuf[:, s * G * V : (s + 1) * G * V],
                    in_=bass.AP(p1v.tensor, p1v[:].offset,
                                [p1v[:].ap[0], [16, G * 16], [1, 16]]),
                    axis=mybir.AxisListType.X,
                    op=OP.add,
                )
            nc.sync.dma_start(out=outb[:], in_=outbuf[:])
    return nc


_NC_CACHE = {}


def _affine_params(knots):
    """Return (k0s, invs) if each dim's knots are (near-)affine, else None."""
    kn = np.asarray(knots, dtype=np.float64)
    k0s, invs = [], []
    for d in range(D):
        kd = kn[d]
        step = (kd[-1] - kd[0]) / (N - 1)
        if step <= 0:
            return None
        fit = kd[0] + step * np.arange(N)
        if not np.allclose(kd, fit, rtol=0, atol=1e-6 * max(1.0, abs(step) * N)):
            return None
        k0s.append(float(kd[0]))
        invs.append(float(1.0 / step))
    return k0s, invs


def _get_nc(mode="general", params=None):
    key = (mode, tuple(params[0]) + tuple(params[1]) if params else None, REPS)
    if key not in _NC_CACHE:
        nc = bacc.Bacc(None, target_bir_lowering=False)
        if mode == "affine":
            _emit_kernel_affine(nc, params[0], params[1])
        else:
            _emit_kernel_general(nc)
        nc.compile()
        _NC_CACHE[key] = nc
    return _NC_CACHE[key]


_PREP_CACHE = {}


def _host_prep(knots, grid, affine=False):
    key = (id(knots), id(grid), affine)
    if key in _PREP_CACHE:
        return _PREP_CACHE[key]
    from numpy.lib.stride_tricks import sliding_window_view

    gp = np.asarray(grid, dtype=np.float32)
    for ax in range(3):
        lo = 2.0 * np.take(gp, 0, axis=ax) - np.take(gp, 1, axis=ax)
        hi = 2.0 * np.take(gp, -1, axis=ax) - np.take(gp, -2, axis=ax)
        gp = np.concatenate(
            [np.expand_dims(lo, ax), gp, np.expand_dims(hi, ax)], axis=ax
        )
    sw = sliding_window_view(gp, (4, 4, 4), axis=(0, 1, 2))
    if affine:
        # rows [47,47,47, k4, v16, (ij)16] = 1024 f32 (4KB): per-k contiguous,
        # v-major inside so ij is innermost everywhere downstream (no strided
        # DVE access in the whole contraction)
        nbr = np.ascontiguousarray(
            sw.transpose(0, 1, 2, 6, 3, 4, 5)  # [.., k, v, i, j]
        ).reshape(NROWS, NB)
        kn = np.asarray(knots, dtype=np.float32)
        knr = np.ascontiguousarray(
            np.broadcast_to(kn.reshape(1, D * N), (P, D * N))
        ).astype(np.float32)
        _PREP_CACHE[key] = (nbr, knr)
        return nbr, knr
    # general path: rows [.., i, j, k, v] + embedded knot windows
    nbr = sw.transpose(0, 1, 2, 4, 5, 6, 3).reshape(NROWS, NB)
    # knot windows per dim: ktw[d][i] = padded_knots[d][i:i+4]
    kn = np.asarray(knots, dtype=np.float32)
    tp = np.concatenate(
        [2.0 * kn[:, :1] - kn[:, 1:2], kn, 2.0 * kn[:, -1:] - kn[:, -2:-1]], axis=1
    )
    ktw = sliding_window_view(tp, 4, axis=1)  # [3, 47, 4]
    gtab = np.empty((NROWS, ROW), dtype=np.float32)
    gtab[:, :NB] = nbr
    kv = gtab[:, NB : NB + 12].reshape(NI, NI, NI, 3, 4)
    kv[:, :, :, 0, :] = ktw[0][:, None, None, :]
    kv[:, :, :, 1, :] = ktw[1][None, :, None, :]
    kv[:, :, :, 2, :] = ktw[2][None, None, :, :]
    gtab[:, NB + 12 :] = 0.0
    knr = np.ascontiguousarray(
        np.broadcast_to(kn.reshape(1, D * N), (P, D * N))
    ).astype(np.float32)
    _PREP_CACHE[key] = (gtab, knr)
    return gtab, knr


def kernel(x, knots, grid):
    x = np.asarray(x, dtype=np.float32)
    params = _affine_params(knots)
    if params is not None:
        gtab, knr = _host_prep(knots, grid, affine=True)
        nc = _get_nc("affine", params)
    else:
        gtab, knr = _host_prep(knots, grid)
        nc = _get_nc()
    in_maps = []
    for c in range(NCORES):
        xs = x[c * BS : (c + 1) * BS]  # [BS, 3]
        xTc = np.ascontiguousarray(
            xs.reshape(T, P, D).transpose(1, 2, 0).reshape(P, D * T)
        )  # [p, d*T + t] = x[t*P + p, d]
        xknc = np.concatenate([xTc, knr], axis=1)
        in_maps.append({"xkn": xknc, "gtab": gtab})
    res = run_bass_kernel_spmd(nc, in_maps, core_ids=list(range(NCORES)))
    outs = []
    for c in range(NCORES):
        ob = res.results[c]["outb"].reshape(P, T, V)  # [p, t, v]
        outs.append(np.ascontiguousarray(ob.transpose(1, 0, 2)).reshape(BS, V))
    return np.concatenate(outs, axis=0)


# revision 18
# speedup vs baseline: 1.7911x; 1.1106x over previous
"""Trainium2 Bass kernel for non-uniform 3D Catmull-Rom spline interpolation.

Problem: x [131072, 3] query points, knots [3, 48], grid [48,48,48,16]
-> out [131072, 16].

Strategy (data-parallel over the batch across 8 cores):
  Host: pad the grid to [50,50,50,16]; build a replicated gather table
    gtab[(p0, q0, r0), 1040] whose 4160B rows hold the full 4x4x4x16
    neighborhood gp[p0:p0+4, q0:q0+4, r0:r0+4, :] (1024 f32) plus the 12
    knot-window values [tm1,t0,t1,t2] per dim (keyed by p0/q0/r0), padded
    to 1040 f32.  A query's entire working set is then ONE contiguous 4KB+
    row -> one [128,1]-index indirect DMA per 128-query tile (the only
    reliably-ordered indirect-DMA shape on TRN2 SWDGE; >=512B descriptors
    run at full DMA rate).
  Device (per core, 16384 queries = 128 tiles of 128 queries on partitions):
    - searchsorted per dim via 47 fused (x >= knot_j) + acc ops (batched
      over all queries)
    - row index = idx0*47*47 + idx1*47 + idx2 -> gidx [128,128] int32
    - per supertile of G tiles: G gathers, Catmull-Rom weights from the
      embedded knot windows (batched [128,G] DVE ops), then per-tile
      contraction k -> j -> i with tensor_scalar / scalar_tensor_tensor
      fused multiply-adds using per-partition scalars (per-query weights)
"""

import numpy as np

import concourse.bass as bass
import concourse.bacc as bacc
import concourse.tile as tile
from concourse import mybir
from concourse.bass_utils import run_bass_kernel_spmd

# ---- problem constants (hardcoded per harness contract) ----
B, D, N, V = 131072, 3, 48, 16
NCORES = 8
BS = B // NCORES  # 16384 queries per core
P = 128           # partitions
T = BS // P       # 128 tiles of 128 queries
NP = N + 2        # 50 (padded grid extent)
NI = N - 1        # 47 interval starts (idx in [0, 46])
NB = 4 * 4 * 4 * V       # 1024 f32 neighborhood per query
ROW = NB + 16            # + [3 dims x 4 knot vals] + 4 pad = 1040 f32 (4160B)
NROWS = NI * NI * NI     # 103823 gather-table rows
G = 16            # tiles per supertile (weight-batching granularity)
NST = T // G      # supertiles

F32 = mybir.dt.float32
I32 = mybir.dt.int32
OP = mybir.AluOpType

DEBUG = False
REPS = 1  # phase-B repetitions (differential HW timing; harness uses 1)


def _emit_kernel_general(nc: bass.Bass):
    xkn = nc.dram_tensor("xkn", [P, D * T + D * N], F32, kind="ExternalInput")
    gtab = nc.dram_tensor("gtab", [NROWS, ROW], F32, kind="ExternalInput")
    outb = nc.dram_tensor("outb", [P, T * V], F32, kind="ExternalOutput")
    if DEBUG:
        dbg_gidx = nc.dram_tensor("dbg_gidx", [P, T], I32, kind="ExternalOutput")
        dbg_nb = nc.dram_tensor("dbg_nb", [P, G * ROW], F32, kind="ExternalOutput")
        dbg_w = nc.dram_tensor("dbg_w", [P, D * 4 * G], F32, kind="ExternalOutput")

    with tile.TileContext(nc) as tc:
        from contextlib import ExitStack

        with ExitStack() as ctx:
            singles = ctx.enter_context(tc.tile_pool(name="singles", bufs=1))
            nbpool = ctx.enter_context(tc.tile_pool(name="nb", bufs=2 * G))
            wpool = ctx.enter_context(tc.tile_pool(name="wp", bufs=2))
            accpool = ctx.enter_context(tc.tile_pool(name="acc", bufs=4))

            # ---------------- load inputs (one DMA) ----------------
            sb_xk = singles.tile([P, D * T + D * N], F32, tag="sb_xk", name="sb_xk")
            nc.sync.dma_start(out=sb_xk[:], in_=xkn[:])
            sb_x = sb_xk[:, : D * T].rearrange("p (d t) -> p d t", d=D)
            sb_kn = sb_xk[:, D * T :].rearrange("p (d n) -> p d n", d=D)

            # ---------------- phase A: searchsorted + row index ----------------
            idx_f = [
                singles.tile([P, T], F32, tag=f"idx{d}", name=f"idx{d}")
                for d in range(D)
            ]
            for d in range(D):
                xd = sb_x[:, d, :]
                acc = idx_f[d]
                nc.vector.tensor_scalar(
                    out=acc[:], in0=xd, scalar1=sb_kn[:, d, 1:2], scalar2=None,
                    op0=OP.is_ge,
                )
                for j in range(2, N):
                    nc.vector.scalar_tensor_tensor(
                        out=acc[:], in0=xd, scalar=sb_kn[:, d, j : j + 1],
                        in1=acc[:], op0=OP.is_ge, op1=OP.add,
                    )
                nc.vector.tensor_scalar(
                    out=acc[:], in0=acc[:], scalar1=float(NI - 1), scalar2=None,
                    op0=OP.min,
                )

            # base row = idx0*NI*NI + idx1*NI + idx2
            basef = singles.tile([P, T], F32, tag="basef", name="basef")
            nc.vector.scalar_tensor_tensor(
                out=basef[:], in0=idx_f[1][:], scalar=float(NI), in1=idx_f[2][:],
                op0=OP.mult, op1=OP.add,
            )
            nc.vector.scalar_tensor_tensor(
                out=basef[:], in0=idx_f[0][:], scalar=float(NI * NI), in1=basef[:],
                op0=OP.mult, op1=OP.add,
            )
            gidx = singles.tile([P, T], I32, tag="gidx", name="gidx")
            nc.vector.tensor_copy(out=gidx[:], in_=basef[:])
            if DEBUG:
                nc.sync.dma_start(out=dbg_gidx[:], in_=gidx[:])

            # ---------------- phase B: gather + weights + contract ----------
            outbuf = singles.tile([P, T * V], F32, tag="outbuf", name="outbuf")
            for s in range(NST * REPS):
                s = s % NST
                nbs = [
                    nbpool.tile([P, ROW], F32, tag="nb", name=f"nb_{s}_{tl}")
                    for tl in range(G)
                ]
                for tl in range(G):
                    t = s * G + tl
                    nc.gpsimd.indirect_dma_start(
                        out=nbs[tl][:],
                        out_offset=None,
                        in_=gtab[:],
                        in_offset=bass.IndirectOffsetOnAxis(
                            ap=gidx[:, t : t + 1], axis=0
                        ),
                    )
                if DEBUG and s == 0:
                    for tl in range(G):
                        nc.sync.dma_start(
                            out=dbg_nb[:, tl * ROW : (tl + 1) * ROW], in_=nbs[tl][:]
                        )

                # collect the embedded knot windows: kvst[p, tl*16 + (d*4+c)]
                kvst = wpool.tile([P, G * 16], F32, tag="kvst", name="kvst")
                for tl in range(G):
                    nc.vector.tensor_copy(
                        out=kvst[:, tl * 16 : (tl + 1) * 16], in_=nbs[tl][:, NB:]
                    )

                # weights per dim, batched over the supertile: [P, G] ops
                w = [
                    [
                        wpool.tile([P, G], F32, tag=f"w{d}{c}", name=f"w{d}{c}")
                        for c in range(4)
                    ]
                    for d in range(D)
                ]
                scr = [
                    wpool.tile([P, G], F32, tag=f"scr{i}", name=f"scr{i}")
                    for i in range(6)
                ]
                u_t, u2_t, u3_t, s0, s1, s2 = scr
                for d in range(D):
                    xd = sb_x[:, d, s * G : (s + 1) * G]
                    kbase = kvst[:]

                    def kvv(c, _kb=kbase, _d=d):
                        return bass.AP(
                            kvst.tensor,
                            _kb.offset + _d * 4 + c,
                            [_kb.ap[0], [16, G]],
                        )

                    tm1, t0, t1, t2 = kvv(0), kvv(1), kvv(2), kvv(3)
                    dt = s0
                    nc.vector.tensor_sub(out=dt[:], in0=t1, in1=t0)
                    xm = s1
                    nc.vector.tensor_sub(out=xm[:], in0=xd, in1=t0)  # x - t0
                    rcp = s2
                    nc.vector.reciprocal(out=rcp[:], in_=dt[:])
                    nc.vector.tensor_mul(out=u_t[:], in0=xm[:], in1=rcp[:])  # u
                    nc.vector.tensor_mul(out=u2_t[:], in0=u_t[:], in1=u_t[:])
                    nc.vector.tensor_mul(out=u3_t[:], in0=u2_t[:], in1=u_t[:])
                    a_t = s1
                    nc.vector.tensor_sub(out=a_t[:], in0=t1, in1=tm1)
                    nc.vector.reciprocal(out=a_t[:], in_=a_t[:])
                    nc.vector.tensor_mul(out=a_t[:], in0=a_t[:], in1=dt[:])
                    b_t = s2
                    nc.vector.tensor_sub(out=b_t[:], in0=t2, in1=t0)
                    nc.vector.reciprocal(out=b_t[:], in_=b_t[:])
                    nc.vector.tensor_mul(out=b_t[:], in0=b_t[:], in1=dt[:])
                    # h00 = 2u3 - 3u2 + 1 ; h10 = u3 - 2u2 + u ; h11 = u3 - u2
                    h00 = w[d][1]
                    nc.vector.tensor_scalar(
                        out=h00[:], in0=u2_t[:], scalar1=-3.0, scalar2=1.0,
                        op0=OP.mult, op1=OP.add,
                    )
                    nc.vector.scalar_tensor_tensor(
                        out=h00[:], in0=u3_t[:], scalar=2.0, in1=h00[:],
                        op0=OP.mult, op1=OP.add,
                    )
                    h10 = w[d][0]
                    nc.vector.tensor_scalar(
                        out=h10[:], in0=u2_t[:], scalar1=-2.0, scalar2=None,
                        op0=OP.mult,
                    )
                    nc.vector.tensor_add(out=h10[:], in0=h10[:], in1=u_t[:])
                    nc.vector.tensor_add(out=h10[:], in0=h10[:], in1=u3_t[:])
                    h11 = w[d][3]
                    nc.vector.tensor_sub(out=h11[:], in0=u3_t[:], in1=u2_t[:])
                    p1 = w[d][0]
                    nc.vector.tensor_mul(out=p1[:], in0=h10[:], in1=a_t[:])
                    p2 = w[d][3]
                    nc.vector.tensor_mul(out=p2[:], in0=h11[:], in1=b_t[:])
                    # w2 = (p1 - h00) + 1 ; w1 = h00 - p2 ; w0 = -p1 ; w3 = p2
                    nc.vector.tensor_sub(out=w[d][2][:], in0=p1[:], in1=h00[:])
                    nc.vector.tensor_scalar(
                        out=w[d][2][:], in0=w[d][2][:], scalar1=1.0, scalar2=None,
                        op0=OP.add,
                    )
                    nc.vector.tensor_sub(out=w[d][1][:], in0=h00[:], in1=p2[:])
                    nc.vector.tensor_scalar(
                        out=w[d][0][:], in0=p1[:], scalar1=-1.0, scalar2=None,
                        op0=OP.mult,
                    )
                if DEBUG and s == 0:
                    for d in range(D):
                        for c in range(4):
                            nc.sync.dma_start(
                                out=dbg_w[
                                    :, (d * 4 + c) * G : (d * 4 + c + 1) * G
                                ],
                                in_=w[d][c][:],
                            )

                # contraction per tile: layout [i(4)x256, j(4)x64, k(4)x16, v(16)]
                for tl in range(G):
                    t = s * G + tl
                    tview = nbs[tl][:]
                    acc1 = accpool.tile([P, 256], F32, tag="acc1", name="acc1")
                    for k in range(4):
                        kslice = bass.AP(
                            nbs[tl].tensor,
                            tview.offset + k * V,
                            [tview.ap[0], [64, 16], [1, V]],
                        )
                        if k == 0:
                            nc.vector.tensor_scalar(
                                out=acc1[:], in0=kslice,
                                scalar1=w[2][k][:, tl : tl + 1],
                                scalar2=None, op0=OP.mult,
                            )
                        else:
                            nc.vector.scalar_tensor_tensor(
                                out=acc1[:], in0=kslice,
                                scalar=w[2][k][:, tl : tl + 1],
                                in1=acc1[:], op0=OP.mult, op1=OP.add,
                            )
                    acc2 = accpool.tile([P, 64], F32, tag="acc2", name="acc2")
                    for j in range(4):
                        jslice = bass.AP(
                            acc1.tensor,
                            acc1[:].offset + j * V,
                            [acc1[:].ap[0], [64, 4], [1, V]],
                        )
                        if j == 0:
                            nc.vector.tensor_scalar(
                                out=acc2[:], in0=jslice,
                                scalar1=w[1][j][:, tl : tl + 1],
                                scalar2=None, op0=OP.mult,
                            )
                        else:
                            nc.vector.scalar_tensor_tensor(
                                out=acc2[:], in0=jslice,
                                scalar=w[1][j][:, tl : tl + 1],
                                in1=acc2[:], op0=OP.mult, op1=OP.add,
                            )
                    oslice = outbuf[:, t * V : (t + 1) * V]
                    for i in range(4):
                        islice = acc2[:, i * V : (i + 1) * V]
                        if i == 0:
                            nc.vector.tensor_scalar(
                                out=oslice, in0=islice,
                                scalar1=w[0][i][:, tl : tl + 1],
                                scalar2=None, op0=OP.mult,
                            )
                        else:
                            nc.vector.scalar_tensor_tensor(
                                out=oslice, in0=islice,
                                scalar=w[0][i][:, tl : tl + 1],
                                in1=oslice, op0=OP.mult, op1=OP.add,
                            )
            nc.sync.dma_start(out=outb[:], in_=outbuf[:])
    return nc


def _emit_kernel_affine(nc: bass.Bass, k0s, invs):
    """Fast path for affine (uniform-spacing) knots: idx/u/weights are pure
    arithmetic (knot values baked as immediates), rows are exactly 1024 f32
    with layout [ij(16), v(16), k(4)] (k innermost)."""
    xkn = nc.dram_tensor("xkn", [P, D * T + D * N], F32, kind="ExternalInput")
    gtab = nc.dram_tensor("gtab", [NROWS, NB], F32, kind="ExternalInput")
    outb = nc.dram_tensor("outb", [P, T * V], F32, kind="ExternalOutput")

    with tile.TileContext(nc) as tc:
        from contextlib import ExitStack

        with ExitStack() as ctx:
            singles = ctx.enter_context(tc.tile_pool(name="singles", bufs=1))
            nbpool = ctx.enter_context(tc.tile_pool(name="nb", bufs=20))
            accpool = ctx.enter_context(tc.tile_pool(name="acc", bufs=2))
            p1pool = ctx.enter_context(tc.tile_pool(name="p1", bufs=2))

            sb_xk = singles.tile([P, D * T + D * N], F32, tag="sb_xk", name="sb_xk")
            nc.sync.dma_start(out=sb_xk[:], in_=xkn[:])
            sb_x = sb_xk[:, : D * T].rearrange("p (d t) -> p d t", d=D)

            # ---- phase A: idx, u, weights (global, arithmetic) ----
            idx_f = [
                singles.tile([P, T], F32, tag=f"idx{d}", name=f"idx{d}")
                for d in range(D)
            ]
            u_d = [
                singles.tile([P, T], F32, tag=f"u{d}", name=f"u{d}") for d in range(D)
            ]
            scr = [
                singles.tile([P, T], F32, tag=f"sc{i}", name=f"sc{i}") for i in range(4)
            ]
            tA, f_t, u2_t, u3_t = scr
            # wd[d] layout [P, (t,c)]: column t*4+c
            wd = [
                singles.tile([P, T * 4], F32, tag=f"wd{d}", name=f"wd{d}")
                for d in range(D)
            ]
            for d in range(D):
                xd = sb_x[:, d, :]
                nc.vector.tensor_scalar(
                    out=tA[:], in0=xd, scalar1=-float(k0s[d]), scalar2=float(invs[d]),
                    op0=OP.add, op1=OP.mult,
                )
                # floor(t) for t in (-1, 48): round(t - 0.5) via the 2^23
                # magic-number trick (one dual-op instruction)
                nc.vector.tensor_scalar(
                    out=idx_f[d][:], in0=tA[:], scalar1=8388607.5,
                    scalar2=8388608.0, op0=OP.add, op1=OP.subtract,
                )
                nc.vector.tensor_scalar(
                    out=idx_f[d][:], in0=idx_f[d][:], scalar1=float(NI - 1),
                    scalar2=0.0, op0=OP.min, op1=OP.max,
                )
                nc.vector.tensor_sub(out=u_d[d][:], in0=tA[:], in1=idx_f[d][:])

            basef = singles.tile([P, T], F32, tag="basef", name="basef")
            nc.vector.scalar_tensor_tensor(
                out=basef[:], in0=idx_f[1][:], scalar=float(NI), in1=idx_f[2][:],
                op0=OP.mult, op1=OP.add,
            )
            nc.vector.scalar_tensor_tensor(
                out=basef[:], in0=idx_f[0][:], scalar=float(NI * NI), in1=basef[:],
                op0=OP.mult, op1=OP.add,
            )
            gidx = singles.tile([P, T], I32, tag="gidx", name="gidx")
            nc.vector.tensor_copy(out=gidx[:], in_=basef[:])

            for d in range(D):
                u = u_d[d]
                nc.vector.tensor_mul(out=u2_t[:], in0=u[:], in1=u[:])
                nc.vector.tensor_mul(out=u3_t[:], in0=u2_t[:], in1=u[:])
                h00 = idx_f[d]  # reuse (idx no longer needed for this dim)
                nc.vector.tensor_scalar(
                    out=h00[:], in0=u2_t[:], scalar1=-3.0, scalar2=1.0,
                    op0=OP.mult, op1=OP.add,
                )
                nc.vector.scalar_tensor_tensor(
                    out=h00[:], in0=u3_t[:], scalar=2.0, in1=h00[:],
                    op0=OP.mult, op1=OP.add,
                )
                h10 = tA
                nc.vector.tensor_scalar(
                    out=h10[:], in0=u2_t[:], scalar1=-2.0, scalar2=None, op0=OP.mult
                )
                nc.vector.tensor_add(out=h10[:], in0=h10[:], in1=u[:])
                nc.vector.tensor_add(out=h10[:], in0=h10[:], in1=u3_t[:])
                h11 = f_t
                nc.vector.tensor_sub(out=h11[:], in0=u3_t[:], in1=u2_t[:])

                def wcol(c, _d=d):
                    base_ap = wd[_d][:]
                    return bass.AP(wd[_d].tensor, base_ap.offset + c, [base_ap.ap[0], [4, T]])

                # w0 = -0.5*h10 ; w3 = 0.5*h11 ; w1 = h00 - 0.5*h11 ; w2 = 1 - h00 + 0.5*h10
                nc.vector.tensor_scalar(
                    out=wcol(0), in0=h10[:], scalar1=-0.5, scalar2=None, op0=OP.mult
                )
                nc.vector.tensor_scalar(
                    out=wcol(3), in0=h11[:], scalar1=0.5, scalar2=None, op0=OP.mult
                )
                nc.vector.scalar_tensor_tensor(
                    out=wcol(1), in0=h11[:], scalar=-0.5, in1=h00[:],
                    op0=OP.mult, op1=OP.add,
                )
                tmp3 = u2_t
                nc.vector.tensor_scalar(
                    out=tmp3[:], in0=h00[:], scalar1=-1.0, scalar2=1.0,
                    op0=OP.mult, op1=OP.add,
                )
                nc.vector.scalar_tensor_tensor(
                    out=wcol(2), in0=h10[:], scalar=0.5, in1=tmp3[:],
                    op0=OP.mult, op1=OP.add,
                )

            # W12[p, t*16 + i*4 + j] = wd0[t,i] * wd1[t,j]
            w12 = singles.tile([P, T * 16], F32, tag="w12", name="w12")
            for i in range(4):
                nc.vector.tensor_tensor(
                    out=bass.AP(w12.tensor, w12[:].offset + i * 4,
                                [w12[:].ap[0], [16, T], [1, 4]]),
                    in0=bass.AP(wd[1].tensor, wd[1][:].offset,
                                [wd[1][:].ap[0], [4, T], [1, 4]]),
                    in1=bass.AP(wd[0].tensor, wd[0][:].offset + i,
                                [wd[0][:].ap[0], [4, T], [0, 4]]),
                    op=OP.mult,
                )

            # ---- phase B ----
            outbuf = singles.tile([P, T * V], F32, tag="outbuf", name="outbuf")
            for s in range(NST * REPS):
                s = s % NST
                nbs = [
                    nbpool.tile([P, NB], F32, tag="nb", name=f"nb_{s}_{tl}")
                    for tl in range(G)
                ]
                for tl in range(G):
                    t = s * G + tl
                    nc.gpsimd.indirect_dma_start(
                        out=nbs[tl][:],
                        out_offset=None,
                        in_=gtab[:],
                        in_offset=bass.IndirectOffsetOnAxis(
                            ap=gidx[:, t : t + 1], axis=0
                        ),
                    )
                acc = accpool.tile([P, G * 256], F32, tag="acct", name="acct")
                for tl in range(G):
                    t = s * G + tl
                    tv = nbs[tl][:]
                    for k in range(4):
                        kslice = bass.AP(
                            nbs[tl].tensor, tv.offset + k * 256, [tv.ap[0], [1, 256]]
                        )
                        oslice = acc[:, tl * 256 : (tl + 1) * 256]
                        sc = bass.AP(
                            wd[2].tensor, wd[2][:].offset + t * 4 + k,
                            [wd[2][:].ap[0], [1, 1]],
                        )
                        if k == 0:
                            nc.vector.tensor_scalar(
                                out=oslice, in0=kslice, scalar1=sc, scalar2=None,
                                op0=OP.mult,
                            )
                        else:
                            nc.vector.scalar_tensor_tensor(
                                out=oslice, in0=kslice, scalar=sc, in1=oslice,
                                op0=OP.mult, op1=OP.add,
                            )
                # stage 2 (batched over supertile):
                # acc layout is (t, v, ij); P1v[t, v, ij] = acc * W12[t, ij]
                p1v = p1pool.tile([P, G * 256], F32, tag="p1v", name="p1v")
                nc.vector.tensor_tensor(
                    out=bass.AP(p1v.tensor, p1v[:].offset,
                                [p1v[:].ap[0], [256, G], [16, 16], [1, 16]]),
                    in0=bass.AP(acc.tensor, acc[:].offset,
                                [acc[:].ap[0], [256, G], [16, 16], [1, 16]]),
                    in1=bass.AP(w12.tensor, w12[:].offset + s * G * 16,
                                [w12[:].ap[0], [16, G], [0, 16], [1, 16]]),
                    op=OP.mult,
                )
                nc.vector.tensor_reduce(
                    out=outbuf[:, s * G * V : (s + 1) * G * V],
                    in_=bass.AP(p1v.tensor, p1v[:].offset,
                                [p1v[:].ap[0], [16, G * 16], [1, 16]]),
                    axis=mybir.AxisListType.X,
                    op=OP.add,
                )
            nc.sync.dma_start(out=outb[:], in_=outbuf[:])
    return nc


_NC_CACHE = {}


def _affine_params(knots):
    """Return (k0s, invs) if each dim's knots are (near-)affine, else None."""
    kn = np.asarray(knots, dtype=np.float64)
    k0s, invs = [], []
    for d in range(D):
        kd = kn[d]
        step = (kd[-1] - kd[0]) / (N - 1)
        if step <= 0:
            return None
        fit = kd[0] + step * np.arange(N)
        if not np.allclose(kd, fit, rtol=0, atol=1e-6 * max(1.0, abs(step) * N)):
            return None
        k0s.append(float(kd[0]))
        invs.append(float(1.0 / step))
    return k0s, invs


def _get_nc(mode="general", params=None):
    key = (mode, tuple(params[0]) + tuple(params[1]) if params else None, REPS)
    if key not in _NC_CACHE:
        nc = bacc.Bacc(None, target_bir_lowering=False)
        if mode == "affine":
            _emit_kernel_affine(nc, params[0], params[1])
        else:
            _emit_kernel_general(nc)
        nc.compile()
        _NC_CACHE[key] = nc
    return _NC_CACHE[key]


_PREP_CACHE = {}


def _host_prep(knots, grid, affine=False):
    key = (id(knots), id(grid), affine)
    if key in _PREP_CACHE:
        return _PREP_CACHE[key][:2]
    from numpy.lib.stride_tricks import sliding_window_view

    gp = np.asarray(grid, dtype=np.float32)
    for ax in range(3):
        lo = 2.0 * np.take(gp, 0, axis=ax) - np.take(gp, 1, axis=ax)
        hi = 2.0 * np.take(gp, -1, axis=ax) - np.take(gp, -2, axis=ax)
        gp = np.concatenate(
            [np.expand_dims(lo, ax), gp, np.expand_dims(hi, ax)], axis=ax
        )
    sw = sliding_window_view(gp, (4, 4, 4), axis=(0, 1, 2))
    if affine:
        # rows [47,47,47, k4, v16, (ij)16] = 1024 f32 (4KB): per-k contiguous,
        # v-major inside so ij is innermost everywhere downstream (no strided
        # DVE access in the whole contraction)
        nbr = np.ascontiguousarray(
            sw.transpose(0, 1, 2, 6, 3, 4, 5)  # [.., k, v, i, j]
        ).reshape(NROWS, NB)
        kn = np.asarray(knots, dtype=np.float32)
        knr = np.ascontiguousarray(
            np.broadcast_to(kn.reshape(1, D * N), (P, D * N))
        ).astype(np.float32)
        # keep refs to knots/grid so their ids stay valid for the cache key
        _PREP_CACHE[key] = (nbr, knr, knots, grid)
        return nbr, knr
    # general path: rows [.., i, j, k, v] + embedded knot windows
    nbr = sw.transpose(0, 1, 2, 4, 5, 6, 3).reshape(NROWS, NB)
    # knot windows per dim: ktw[d][i] = padded_knots[d][i:i+4]
    kn = np.asarray(knots, dtype=np.float32)
    tp = np.concatenate(
        [2.0 * kn[:, :1] - kn[:, 1:2], kn, 2.0 * kn[:, -1:] - kn[:, -2:-1]], axis=1
    )
    ktw = sliding_window_view(tp, 4, axis=1)  # [3, 47, 4]
    gtab = np.empty((NROWS, ROW), dtype=np.float32)
    gtab[:, :NB] = nbr
    kv = gtab[:, NB : NB + 12].reshape(NI, NI, NI, 3, 4)
    kv[:, :, :, 0, :] = ktw[0][:, None, None, :]
    kv[:, :, :, 1, :] = ktw[1][None, :, None, :]
    kv[:, :, :, 2, :] = ktw[2][None, None, :, :]
    gtab[:, NB + 12 :] = 0.0
    knr = np.ascontiguousarray(
        np.broadcast_to(kn.reshape(1, D * N), (P, D * N))
    ).astype(np.float32)
    _PREP_CACHE[key] = (gtab, knr, knots, grid)
    return gtab, knr


def kernel(x, knots, grid):
    x = np.asarray(x, dtype=np.float32)
    params = _affine_params(knots)
    if params is not None:
        gtab, knr = _host_prep(knots, grid, affine=True)
        nc = _get_nc("affine", params)
    else:
        gtab, knr = _host_prep(knots, grid)
        nc = _get_nc()
    in_maps = []
    for c in range(NCORES):
        xs = x[c * BS : (c + 1) * BS]  # [BS, 3]
        xTc = np.ascontiguousarray(
            xs.reshape(T, P, D).transpose(1, 2, 0).reshape(P, D * T)
        )  # [p, d*T + t] = x[t*P + p, d]
        xknc = np.concatenate([xTc, knr], axis=1)
        in_maps.append({"xkn": xknc, "gtab": gtab})
    res = run_bass_kernel_spmd(nc, in_maps, core_ids=list(range(NCORES)))
    outs = []
    for c in range(NCORES):
        ob = res.results[c]["outb"].reshape(P, T, V)  # [p, t, v]
        outs.append(np.ascontiguousarray(ob.transpose(1, 0, 2)).reshape(BS, V))
    return np.concatenate(outs, axis=0)
